# revision 1
# baseline (speedup 1.0000x reference)
"""Self-contained TRN2 Bass kernel for the 2-layer multi-head GAT problem.

kernel(**inputs) -> np.ndarray [100000, 40] float32 (log_softmax outputs).

Strategy: dst-sharded graph parallelism across 8 NeuronCores. Dense phases
compute packed bf16 node tables (msg | a_s | a_d) that are AllGathered; edge
phases gather source rows with dma_gather (int16 idx, 4 src blocks, runtime
valid-counts via reg_load), aggregate per 128-dst tile via one-hot matmuls in
PSUM, with the segment softmax folded into a single normalize at the end
(exp-shift constant M is softmax-invariant).

v2 layout/scheduling notes:
 - layer-1 msg columns are head-minor (col = w*H + h) so the per-edge ee
   multiply broadcasts over a middle axis and runs in DVE 2x mode.
 - the dst one-hot S is built per 2-tile group as S_alt[e, d, c] (chunk
   innermost) from a repeated-iota bf16 constant, also 2x mode.
 - gather calls stay per (tile, block): the SWDGE descriptor ring caps a
   single call at ~1024 descriptors, so multi-tile merged calls overflow it.
 - log_softmax's Ln runs once at the end over all tiles (avoids act-table
   thrash), and the output leaves in one DMA.
"""

import math
import sys
from contextlib import ExitStack
from dataclasses import dataclass, field

import numpy as np

sys.path.insert(0, "/opt/trn_rl_repo")

import concourse.bacc as bacc
import concourse.bass as bass
import concourse.tile as tile
from concourse import mybir
from concourse.masks import make_identity

F32 = mybir.dt.float32
BF16 = mybir.dt.bfloat16
I16 = mybir.dt.int16


@dataclass
class Cfg:
    n_nodes: int = 100000
    f_in: int = 128
    hid: int = 32
    heads: int = 8
    classes: int = 40
    n_cores: int = 8
    tiles_per_core: int = 100
    n_blocks: int = 4
    group: int = 2         # tiles per gather-call group
    m1: float = 16.0       # exp-shift layer 1
    m2: float = 16.0       # exp-shift layer 2
    neg_slope: float = 0.2
    ablate: str = ""
    repeat: int = 1
    c_tb: object = None    # [T, B] chunks per (tile, block)
    # derived host bookkeeping (set in prep_host_data)
    grp_chunks: object = None   # [NG] chunks per group
    grp_choff: object = None    # [NG] chunk offset of group in global stream
    call_caps: object = None    # [NG, G, B] capacity (idx) per call
    call_ioff: object = None    # [NG, G, B] idx16 col offset per call
    call_coff: object = None    # [NG, G, B] chunk offset of call within group
    tile_runs: object = None    # [T] list of (chunk_pos_in_group, count) runs
    chunk_tile: object = None   # global chunk -> tile-in-group (0..G-1)
    total_idx16: int = 0
    total_chunks: int = 0

    @property
    def d1(self):
        return self.heads * self.hid

    @property
    def n_pad(self):
        return self.n_cores * self.tiles_per_core * 128

    @property
    def nodes_per_core(self):
        return self.tiles_per_core * 128

    @property
    def block_rows(self):
        assert self.n_pad % self.n_blocks == 0
        return self.n_pad // self.n_blocks

    @property
    def block_rows_h(self):
        return self.n_pad // self.n_blocks

    @property
    def n_groups(self):
        assert self.tiles_per_core % self.group == 0
        return self.tiles_per_core // self.group

    @property
    def row1(self):
        # bf16 cols of table1 row; stride must be mult of 128 (256B)
        need = self.d1 + 2 * self.heads
        return ((need + 127) // 128) * 128

    @property
    def row2(self):
        need = self.hid + 2  # h2 | a_s2 | a_d2
        return ((need + 127) // 128) * 128

    @property
    def cbg(self):
        # max chunks per group (tile alloc size)
        return int(self.grp_chunks.max())


def degree_balance_perm(dst: np.ndarray, cfg: Cfg) -> np.ndarray:
    """pi[old_id] = new_id; in-degrees balanced across 128-node tiles via
    snake round-robin over tiles in descending-degree order. Vectorized."""
    n, npad = cfg.n_nodes, cfg.n_pad
    deg = np.bincount(dst, minlength=n).astype(np.int64) + 1
    order = np.argsort(-deg, kind="stable")
    n_tiles = npad // 128
    i = np.arange(n, dtype=np.int64)
    rnd = i // n_tiles
    pos = i % n_tiles
    tl = np.where(rnd % 2 == 0, pos, n_tiles - 1 - pos)
    slot = rnd
    assert slot.max() < 128
    pi = np.empty(npad, dtype=np.int64)
    pi[order] = tl * 128 + slot
    # pad ids -> remaining slots
    used = np.zeros(npad, dtype=bool)
    used[pi[:n]] = True
    pi[n:] = np.flatnonzero(~used)
    return pi


@dataclass
class HostData:
    perm: np.ndarray
    inv_perm: np.ndarray
    per_core: list  # dict of input arrays per core


def wrap16(idx_1d: np.ndarray) -> np.ndarray:
    """[n] -> [16, n/16] wrapped (j at [j%16, j//16]), tiled to [128, n/16]."""
    n = idx_1d.shape[0]
    assert n % 16 == 0
    w = idx_1d.reshape(n // 16, 16).T.copy()  # [16, n/16]
    return np.tile(w, (8, 1))  # [128, n/16]


def interleave_cols(H, C):
    """perm p: new col j=w*H+h takes old col h*C+w. Returns old-index array
    such that new[:, j] = old[:, p[j]]."""
    p = np.empty(H * C, dtype=np.int64)
    for w in range(C):
        for h in range(H):
            p[w * H + h] = h * C + w
    return p


def prep_host_data(cfg: Cfg, inputs: dict) -> HostData:
    n, npad = cfg.n_nodes, cfg.n_pad
    H, C = cfg.heads, cfg.hid
    T, B, G = cfg.tiles_per_core, cfg.n_blocks, cfg.group
    NG = cfg.n_groups
    x = np.asarray(inputs["x"], dtype=np.float32)
    ei = np.asarray(inputs["edge_index"])
    src0 = ei[0].astype(np.int64)
    dst0 = ei[1].astype(np.int64)
    loops = np.arange(n, dtype=np.int64)
    src0 = np.concatenate([src0, loops])
    dst0 = np.concatenate([dst0, loops])

    perm = degree_balance_perm(dst0, cfg)
    inv_perm = np.argsort(perm)
    src = perm[src0]
    dst = perm[dst0]

    # --- weights ---
    W1 = np.asarray(inputs["W1"], dtype=np.float64)      # [F, H*C]
    att_s1 = np.asarray(inputs["att_s1"], dtype=np.float64)  # [H, C]
    att_d1 = np.asarray(inputs["att_d1"], dtype=np.float64)
    b1 = np.asarray(inputs["b1"], dtype=np.float32)
    W2 = np.asarray(inputs["W2"], dtype=np.float64)      # [H*C, C]
    att_s2 = np.asarray(inputs["att_s2"], dtype=np.float64)  # [1, C]
    att_d2 = np.asarray(inputs["att_d2"], dtype=np.float64)
    b2 = np.asarray(inputs["b2"], dtype=np.float32)
    Wout = np.asarray(inputs["Wout"], dtype=np.float32)
    bout = np.asarray(inputs["bout"], dtype=np.float32)

    pcols = interleave_cols(H, C)  # head-minor msg column order
    Ws1 = np.zeros((cfg.f_in, H), dtype=np.float64)
    Wd1 = np.zeros((cfg.f_in, H), dtype=np.float64)
    for h in range(H):
        Ws1[:, h] = W1[:, h * C:(h + 1) * C] @ att_s1[h]
        Wd1[:, h] = W1[:, h * C:(h + 1) * C] @ att_d1[h]
    W1msg = W1[:, pcols]  # [F, 256] head-minor
    W1ext = np.concatenate([W1msg, Ws1, Wd1], axis=1).astype(np.float32)
    b1p = b1[pcols]
    # W2ext rows follow the head-minor order of layer-1 msg cols
    Ws2 = W2 @ att_s2[0]
    Wd2 = W2 @ att_d2[0]
    W2ext = np.concatenate([W2, Ws2[:, None], Wd2[:, None]], axis=1)[pcols].astype(np.float32)

    # --- per-core edge prep ---
    npc = cfg.nodes_per_core
    core_of_edge = dst // npc
    sblock = src // cfg.block_rows
    sloc_all = (src % cfg.block_rows).astype(np.int16)
    per_core = []
    for k in range(cfg.n_cores):
        m = core_of_edge == k
        es, ed = src[m], dst[m]
        eb, el = sblock[m], sloc_all[m]
        tile_of = (ed - k * npc) // 128
        block_of = eb
        key = tile_of * B + block_of
        order = np.argsort(key, kind="stable")
        es, ed, el = es[order], ed[order], el[order]
        counts = np.bincount(key[order], minlength=T * B)
        per_core.append(dict(es=es, ed=ed, el=el, counts=counts, k=k))

    all_counts = np.stack([pc["counts"] for pc in per_core])  # [cores, T*B]
    c_tb = ((all_counts.max(axis=0) + 127) // 128).astype(np.int64)
    c_tb = np.maximum(c_tb, 1).reshape(T, B)
    cfg.c_tb = c_tb

    # --- group/call bookkeeping ---
    # per group gi: chunk stream layout = [b0: t0..tG-1 | b1: t0.. | ...]
    # gather calls stay per (t, b): the SWDGE desc ring caps one call at
    # ~1024 descriptors, so merged multi-tile calls crash the device.
    grp_chunks = np.zeros(NG, dtype=np.int64)
    call_caps = np.zeros((NG, G, B), dtype=np.int64)   # idxs per call (padded)
    call_coff = np.zeros((NG, G, B), dtype=np.int64)   # chunk offset in group
    for gi in range(NG):
        off = 0
        for b in range(B):
            for ti in range(G):
                t = gi * G + ti
                call_coff[gi, ti, b] = off
                call_caps[gi, ti, b] = int(c_tb[t, b]) * 128
                off += int(c_tb[t, b])
        grp_chunks[gi] = off
    grp_choff = np.concatenate([[0], np.cumsum(grp_chunks)])[:-1]
    total_chunks = int(grp_chunks.sum())
    call_ioff = np.zeros((NG, G, B), dtype=np.int64)
    acc = 0
    for gi in range(NG):
        for b in range(B):
            for ti in range(G):
                call_ioff[gi, ti, b] = acc
                acc += int(call_caps[gi, ti, b]) // 16
    total_idx16 = acc

    # per-tile chunk runs within its group: [(pos_in_group, nchunks), ...] per block
    tile_runs = []
    chunk_tile = np.zeros(total_chunks, dtype=np.int64)
    for t in range(T):
        gi, ti = t // G, t % G
        runs = []
        for b in range(B):
            pos = int(call_coff[gi, ti, b])
            runs.append((pos, int(c_tb[t, b])))
            chunk_tile[grp_choff[gi] + pos: grp_choff[gi] + pos + c_tb[t, b]] = ti
        tile_runs.append(runs)

    cfg.grp_chunks = grp_chunks
    cfg.grp_choff = grp_choff
    cfg.call_caps = call_caps
    cfg.call_ioff = call_ioff
    cfg.call_coff = call_coff
    cfg.tile_runs = tile_runs
    cfg.chunk_tile = chunk_tile
    cfg.total_idx16 = total_idx16
    cfg.total_chunks = total_chunks

    # --- per-core idx / dstloc streams ---
    per_core_arrays = []
    for pc in per_core:
        es, ed, el, counts, k = pc["es"], pc["ed"], pc["el"], pc["counts"], pc["k"]
        starts = np.concatenate([[0], np.cumsum(counts)])
        idx_flat = np.full(total_idx16 * 16, -1, dtype=np.int16)
        dstloc = np.full(total_chunks * 128, -1.0, dtype=np.float32)
        ncnt = np.zeros(NG * G * B, dtype=np.int32)
        for gi in range(NG):
            for b in range(B):
                for ti in range(G):
                    t = gi * G + ti
                    gidx = t * B + b
                    s0, s1 = starts[gidx], starts[gidx + 1]
                    cnt = int(s1 - s0)
                    cap = int(call_caps[gi, ti, b])
                    assert cnt <= cap
                    ibase = int(call_ioff[gi, ti, b]) * 16
                    sbase = (int(grp_choff[gi]) + int(call_coff[gi, ti, b])) * 128
                    idx_flat[ibase: ibase + cnt] = el[s0:s1]
                    dl = (ed[s0:s1] - k * npc - t * 128).astype(np.float32)
                    dstloc[sbase: sbase + cnt] = dl
                    ncnt[(gi * G + ti) * B + b] = cnt
        # wrap idx per call into [128, total_idx16]
        idx_wrapped = np.zeros((128, total_idx16), dtype=np.int16)
        for gi in range(NG):
            for b in range(B):
                for ti in range(G):
                    capi = int(call_caps[gi, ti, b])
                    base = int(call_ioff[gi, ti, b])
                    w = wrap16(idx_flat[base * 16: base * 16 + capi])
                    idx_wrapped[:, base:base + capi // 16] = w
        import ml_dtypes
        dl3 = dstloc.reshape(total_chunks, 128)
        dstloc_part = np.ascontiguousarray(dl3.T).astype(ml_dtypes.bfloat16)
        arrs = dict(
            idx=idx_wrapped,
            dstloc_p=dstloc_part,
            ncnt=ncnt.reshape(1, -1),
        )
        per_core_arrays.append(arrs)

    # --- dense inputs per core ---
    import ml_dtypes
    x_pad = np.zeros((npad, cfg.f_in), dtype=np.float32)
    x_pad[perm[:n]] = x
    xT = x_pad.T.astype(ml_dtypes.bfloat16)  # [F, npad]

    cbg = cfg.cbg
    iota_rep = np.zeros((1, 128 * cbg), dtype=np.float32)
    iota_rep[0] = np.repeat(np.arange(128, dtype=np.float32), cbg)
    iota_rep = iota_rep.astype(ml_dtypes.bfloat16)

    ck = min(128, cfg.d1)
    nk = cfg.d1 // ck
    W2chunk = np.ascontiguousarray(
        W2ext.reshape(nk, ck, C + 2).transpose(1, 0, 2)).reshape(ck, -1)
    for k, arrs in enumerate(per_core_arrays):
        arrs["xT"] = np.ascontiguousarray(xT[:, k * npc:(k + 1) * npc])
        arrs["W1ext"] = W1ext.astype(ml_dtypes.bfloat16)
        arrs["W2ext"] = W2chunk.astype(ml_dtypes.bfloat16)
        arrs["Wout"] = Wout.astype(ml_dtypes.bfloat16)
        arrs["b1"] = b1p[None, :].astype(np.float32)
        arrs["b2"] = b2[None, :].astype(np.float32)
        arrs["bout"] = bout[None, :].astype(np.float32)
        arrs["iota_rep"] = iota_rep

    return HostData(perm=perm, inv_perm=inv_perm, per_core=per_core_arrays)


# ============================== device program ==============================

def build_program(cfg: Cfg, debug: bool = False):
    nc = bacc.Bacc("TRN2", target_bir_lowering=False, debug=debug,
                   num_devices=cfg.n_cores, num_swdge_queues=4)
    T, B, H, C, G = cfg.tiles_per_core, cfg.n_blocks, cfg.heads, cfg.hid, cfg.group
    NG = cfg.n_groups
    D1 = cfg.d1
    npc, npad = cfg.nodes_per_core, cfg.n_pad
    cbg = cfg.cbg
    row1, row2 = cfg.row1, cfg.row2
    F = cfg.f_in
    groups8 = [list(range(cfg.n_cores))]

    # ---- inputs ----
    xT = nc.dram_tensor("xT", [F, npc], BF16, kind="ExternalInput")
    W1ext = nc.dram_tensor("W1ext", [F, D1 + 2 * H], BF16, kind="ExternalInput")
    ck = min(128, D1)
    nk = D1 // ck
    W2ext = nc.dram_tensor("W2ext", [ck, nk * (C + 2)], BF16, kind="ExternalInput")
    Wout = nc.dram_tensor("Wout", [C, cfg.classes], BF16, kind="ExternalInput")
    b1 = nc.dram_tensor("b1", [1, D1], F32, kind="ExternalInput")
    b2 = nc.dram_tensor("b2", [1, C], F32, kind="ExternalInput")
    bout = nc.dram_tensor("bout", [1, cfg.classes], F32, kind="ExternalInput")
    iota_rep = nc.dram_tensor("iota_rep", [1, 128 * cbg], BF16, kind="ExternalInput")
    idx = nc.dram_tensor("idx", [128, cfg.total_idx16], I16, kind="ExternalInput")
    ncnt = nc.dram_tensor("ncnt", [1, NG * G * B], mybir.dt.int32, kind="ExternalInput")
    dstloc_p = nc.dram_tensor("dstloc_p", [128, cfg.total_chunks], BF16, kind="ExternalInput")

    # ---- internal / output ----
    tab1_sh = nc.dram_tensor("tab1_sh", [npc, row1], BF16)
    tab1 = nc.dram_tensor("tab1", [npad, row1], BF16, addr_space="Shared")
    tab2_sh = nc.dram_tensor("tab2_sh", [npc, row2], BF16)
    tab2 = nc.dram_tensor("tab2", [npad, row2], BF16, addr_space="Shared")
    out = nc.dram_tensor("out", [npc, cfg.classes], F32, kind="ExternalOutput")

    with tile.TileContext(nc, num_cores=cfg.n_cores) as tc, ExitStack() as ctx:
        consts = ctx.enter_context(tc.tile_pool(name="consts", bufs=1))

        # resident constants
        W1e_sb = consts.tile([F, D1 + 2 * H], BF16)
        nc.sync.dma_start(out=W1e_sb, in_=W1ext[:])
        W2e_sb = consts.tile([ck, nk, C + 2], BF16)
        nc.sync.dma_start(out=W2e_sb, in_=W2ext[:].rearrange("p (a c) -> p a c", a=nk))
        Wout_sb = consts.tile([C, cfg.classes], BF16)
        nc.sync.dma_start(out=Wout_sb, in_=Wout[:])
        b1_sb = consts.tile([128, D1], F32)
        nc.sync.dma_start(out=b1_sb, in_=bass.AP(
            tensor=b1.ap().tensor, offset=0, ap=[[0, 128], [1, D1]]))
        b2_sb = consts.tile([128, C], F32)
        nc.sync.dma_start(out=b2_sb, in_=bass.AP(
            tensor=b2.ap().tensor, offset=0, ap=[[0, 128], [1, C]]))
        bout_sb = consts.tile([128, cfg.classes], F32)
        nc.sync.dma_start(out=bout_sb, in_=bass.AP(
            tensor=bout.ap().tensor, offset=0, ap=[[0, 128], [1, cfg.classes]]))
        iota_sb = consts.tile([128, 128 * cbg], BF16)
        nc.sync.dma_start(out=iota_sb, in_=bass.AP(
            tensor=iota_rep.ap().tensor, offset=0, ap=[[0, 128], [1, 128 * cbg]]))
        ident_sb = consts.tile([128, 128], BF16)
        make_identity(nc, ident_sb)
        zero_b = consts.tile([128, 1], F32)
        nc.vector.memset(zero_b, 0.0)
        mneg1_b = consts.tile([128, 1], F32)
        nc.vector.memset(mneg1_b, -cfg.m1)
        mneg2_b = consts.tile([128, 1], F32)
        nc.vector.memset(mneg2_b, -cfg.m2)
        idx_sb = consts.tile([128, cfg.total_idx16], I16)
        nc.sync.dma_start(out=idx_sb, in_=idx[:])
        ncnt_sb = consts.tile([1, NG * G * B], mybir.dt.int32)
        nc.sync.dma_start(out=ncnt_sb, in_=ncnt[:])
        gcnt_regs = [nc.gpsimd.alloc_register(f"gcnt{i}") for i in range(2 * G * B)]
        dlp_sb = consts.tile([128, cfg.total_chunks], BF16)
        nc.sync.dma_start(out=dlp_sb, in_=dstloc_p[:])
        a_d1_sb = consts.tile([128, T, 1, H], BF16)
        a_d2_sb = consts.tile([128, T, 1, 1], BF16)
        zs_buf = consts.tile([128, T, cfg.classes], BF16)
        ss_buf = consts.tile([128, T], F32)

        for _rep in range(cfg.repeat):
            # ---------------- phase A: dense + table1 ----------------
            with tc.tile_pool(name="phA", bufs=3) as pa, \
                 tc.tile_pool(name="phA_x", bufs=1) as pax, \
                 tc.tile_pool(name="phA_ps", bufs=2, space="PSUM") as pap:
                xT_sb = pax.tile([F, npc], BF16)
                nc.sync.dma_start(out=xT_sb, in_=xT[:])
                for t in range(T):
                    ps = pap.tile([128, D1 + 2 * H], F32, tag="psA")
                    nc.tensor.matmul(out=ps, lhsT=xT_sb[:, t * 128:(t + 1) * 128],
                                     rhs=W1e_sb[:], start=True, stop=True)
                    row = pa.tile([128, row1], BF16, tag="rowA")
                    if row1 > D1 + 2 * H:
                        nc.vector.memset(row[:, D1 + 2 * H:], 0.0)
                    nc.scalar.copy(row[:, 0:D1 + 2 * H], ps[:])
                    nc.vector.tensor_copy(a_d1_sb[:, t, 0, :], ps[:, D1 + H:D1 + 2 * H])
                    nc.sync.dma_start(out=tab1_sh[t * 128:(t + 1) * 128, :], in_=row[:])

            if cfg.ablate == "nocoll":
                nc.sync.dma_start(out=tab1[0:npc, :], in_=tab1_sh[:])
            else:
                nc.gpsimd.collective_compute(
                    "AllGather", mybir.AluOpType.bypass, groups8,
                    ins=[tab1_sh[:]], outs=[tab1[:]])

            # ---------------- shared edge pipeline ----------------
            def edge_phase(tab, rowN, DN, HN, a_d_sb, mneg_bias, post_tile):
                """DN = msg width, HN = heads. post_tile(t, psG, pools)."""
                with tc.tile_pool(name="phB_g", bufs=2) as pgath, \
                     tc.tile_pool(name="phB_S", bufs=2) as pS, \
                     tc.tile_pool(name="phB_S2", bufs=1) as pS2, \
                     tc.tile_pool(name="phB_m", bufs=1) as pmsg, \
                     tc.tile_pool(name="phB_ps", bufs=2, space="PSUM") as pbp, \
                     tc.tile_pool(name="phB_ep", bufs=2) as pep, \
                     tc.tile_pool(name="phB_psS2", bufs=1, space="PSUM") as pbpS2, \
                     tc.tile_pool(name="phB_psE", bufs=1, space="PSUM") as pbpE, \
                     tc.tile_pool(name="phB_ps2", bufs=2, space="PSUM") as pbp2:
                    for gi in range(NG):
                        cbgi = int(cfg.grp_chunks[gi])
                        choff = int(cfg.grp_choff[gi])
                        g = pgath.tile([128, cbg, rowN], BF16, tag="gath")
                        if gi < 2:
                            nc.vector.memset(g, 0.0)
                        for b in range(B):
                            for ti in range(G):
                                cap = int(cfg.call_caps[gi, ti, b])
                                coff = int(cfg.call_coff[gi, ti, b])
                                ioff = int(cfg.call_ioff[gi, ti, b])
                                ci = (gi * G + ti) * B + b
                                reg = gcnt_regs[(gi % 2) * G * B + ti * B + b]
                                nc.gpsimd.reg_load(reg, ncnt_sb[0:1, ci:ci + 1])
                                nc.gpsimd.dma_gather(
                                    g[:, coff:coff + cap // 128, :],
                                    tab[b * cfg.block_rows:(b + 1) * cfg.block_rows, :],
                                    idx_sb[:, ioff:ioff + cap // 16],
                                    cap, reg, rowN, elem_step=rowN,
                                    queue_num=b % nc.num_swdge_queues)
                        # one-hot S_alt[e, d, c] (chunk innermost; 2x mode)
                        S = pS.tile([128, 128, cbg], BF16, tag="S")
                        nc.vector.tensor_tensor(
                            out=S[:, :, :cbgi],
                            in0=iota_sb[:].rearrange("p (q c) -> p q c", q=128)[:, :, :cbgi],
                            in1=dlp_sb[:, choff:choff + cbgi].unsqueeze(1).broadcast_to(
                                [128, 128, cbgi]),
                            op=mybir.AluOpType.is_equal)
                        # S2 = per-chunk transpose of S (d on partitions)
                        psE = pbpE.tile([128, cbg, 1, HN], F32, tag="psE")
                        half = (cbgi + 3) // 4
                        for hh in range(4):
                            j0 = hh * half
                            j1 = min(cbgi, j0 + half)
                            if j1 <= j0:
                                continue
                            psS2 = pbpS2.tile([128, half, 128], BF16, tag="psS2")
                            for j in range(j0, j1):
                                nc.tensor.transpose(psS2[:, j - j0, :], S[:, :, j],
                                                    ident_sb[:])
                            S2 = pS2.tile([128, half, 128], BF16, tag="S2")
                            nc.scalar.copy(S2[:, 0:j1 - j0, :], psS2[:, 0:j1 - j0, :])
                            for j in range(j0, j1):
                                t = gi * G + int(cfg.chunk_tile[choff + j])
                                nc.tensor.matmul(
                                    out=psE[:, j, :, :], lhsT=S2[:, j - j0, :],
                                    rhs=a_d_sb[:, t, :, :].rearrange("p a b -> p (a b)"),
                                    start=True, stop=True)
                        # es = a_s[src] + a_d[dst] (+ leaky)
                        es = pep.tile([128, cbg, HN], F32, tag="es")
                        nc.vector.tensor_tensor(out=es[:, :cbgi, :],
                                                in0=psE[:, :cbgi, 0, :],
                                                in1=g[:, :cbgi, DN:DN + HN],
                                                op=mybir.AluOpType.add)
                        nc.vector.scalar_tensor_tensor(
                            out=es[:, :cbgi, :], in0=es[:, :cbgi, :],
                            scalar=cfg.neg_slope, in1=es[:, :cbgi, :],
                            op0=mybir.AluOpType.mult, op1=mybir.AluOpType.max)
                        ee = pep.tile([128, cbg, HN], BF16, tag="ee")
                        nc.scalar.activation(ee[:, :cbgi, :], es[:, :cbgi, :],
                                             mybir.ActivationFunctionType.Exp,
                                             bias=mneg_bias[:], scale=1.0)
                        # msgee = [g * ee (head-minor broadcast) | ee]
                        msgee = pmsg.tile([128, cbg, DN + HN], BF16, tag="msgee")
                        if HN > 1:
                            nc.vector.tensor_tensor(
                                out=msgee[:, :cbgi, 0:DN].rearrange(
                                    "p c (w h) -> p c w h", h=HN),
                                in0=g[:, :cbgi, 0:DN].rearrange(
                                    "p c (w h) -> p c w h", h=HN),
                                in1=ee[:, :cbgi, :].unsqueeze(2).broadcast_to(
                                    [128, cbgi, DN // HN, HN]),
                                op=mybir.AluOpType.mult)
                        else:
                            nc.vector.tensor_tensor(
                                out=msgee[:, :cbgi, 0:DN],
                                in0=g[:, :cbgi, 0:DN],
                                in1=ee[:, :cbgi, :].broadcast_to([128, cbgi, DN]),
                                op=mybir.AluOpType.mult)
                        nc.vector.tensor_copy(msgee[:, :cbgi, DN:DN + HN],
                                              ee[:, :cbgi, :])
                        # aggregate per tile over its chunk runs
                        for ti in range(G):
                            t = gi * G + ti
                            runs = cfg.tile_runs[t]
                            chunks = [p for (pos, cnt) in runs
                                      for p in range(pos, pos + cnt)]
                            psG = pbp.tile([128, DN + HN], F32, tag="psG")
                            for jj, j in enumerate(chunks):
                                nc.tensor.matmul(out=psG, lhsT=S[:, :, j],
                                                 rhs=msgee[:, j, :],
                                                 start=(jj == 0),
                                                 stop=(jj == len(chunks) - 1))
                            post_tile(t, psG, pep, pbp2)

            # ---- layer-1 post-tile: divide, +b1, elu, h2 fold ----
            def post1(t, psG, pep, pps):
                den = pep.tile([128, H], F32, tag="den")
                nc.vector.tensor_scalar_add(den, psG[:, D1:D1 + H], 1e-30)
                rden = pep.tile([128, H], F32, tag="rden")
                nc.vector.reciprocal(rden, den[:])
                z = pep.tile([128, D1], F32, tag="z")
                nc.vector.tensor_tensor(
                    out=z[:].rearrange("p (w h) -> p w h", h=H),
                    in0=psG[:, 0:D1].rearrange("p (w h) -> p w h", h=H),
                    in1=rden[:].unsqueeze(1).broadcast_to([128, C, H]),
                    op=mybir.AluOpType.mult)
                nc.vector.tensor_tensor(out=z, in0=z[:], in1=b1_sb[:],
                                        op=mybir.AluOpType.add)
                zm = pep.tile([128, D1], F32, tag="zm")
                nc.vector.tensor_scalar_min(zm, z[:], 0.0)
                em = pep.tile([128, D1], F32, tag="em")
                nc.scalar.activation(em, zm[:], mybir.ActivationFunctionType.Exp,
                                     bias=zero_b[:])
                zp = pep.tile([128, D1], F32, tag="zp")
                nc.vector.tensor_scalar_max(zp, z[:], 0.0)
                elu_bf = pep.tile([128, D1], BF16, tag="elubf")
                nc.vector.scalar_tensor_tensor(
                    out=elu_bf, in0=em[:], scalar=-1.0, in1=zp[:],
                    op0=mybir.AluOpType.add, op1=mybir.AluOpType.add)
                # h2 = eluT @ W2ext
                eluT = pep.tile([ck, nk, 128], BF16, tag="eluT")
                for kk in range(nk):
                    psT = pps.tile([ck, 128], BF16, tag="psmisc")
                    nc.tensor.transpose(psT, elu_bf[:, kk * ck:(kk + 1) * ck], ident_sb[:])
                    nc.scalar.copy(eluT[:, kk, :], psT[:])
                psH2 = pps.tile([128, C + 2], F32, tag="psmisc")
                for kk in range(nk):
                    nc.tensor.matmul(out=psH2, lhsT=eluT[:, kk, :], rhs=W2e_sb[:, kk, :],
                                     start=(kk == 0), stop=(kk == nk - 1))
                row = pep.tile([128, row2], BF16, tag="rowC")
                if row2 > C + 2:
                    nc.vector.memset(row[:, C + 2:], 0.0)
                nc.scalar.copy(row[:, 0:C + 2], psH2[:])
                nc.vector.tensor_copy(a_d2_sb[:, t, 0, :], psH2[:, C + 1:C + 2])
                nc.sync.dma_start(out=tab2_sh[t * 128:(t + 1) * 128, :], in_=row[:])

            edge_phase(tab1, row1, D1, H, a_d1_sb, mneg1_b, post1)

            if cfg.ablate == "nocoll":
                nc.sync.dma_start(out=tab2[0:npc, :], in_=tab2_sh[:])
            else:
                nc.gpsimd.collective_compute(
                    "AllGather", mybir.AluOpType.bypass, groups8,
                    ins=[tab2_sh[:]], outs=[tab2[:]])

            # ---- layer-2 post-tile: divide, +b2, elu, head, partial smax ----
            def post2(t, psG, pep, pps):
                den = pep.tile([128, 1], F32, tag="den2")
                nc.vector.tensor_scalar_add(den, psG[:, C:C + 1], 1e-30)
                rden = pep.tile([128, 1], F32, tag="rden2")
                nc.vector.reciprocal(rden, den[:])
                z = pep.tile([128, C], F32, tag="z2")
                nc.vector.tensor_scalar(out=z, in0=psG[:, 0:C], scalar1=rden[:],
                                        scalar2=None, op0=mybir.AluOpType.mult)
                nc.vector.tensor_tensor(out=z, in0=z[:], in1=b2_sb[:],
                                        op=mybir.AluOpType.add)
                zm = pep.tile([128, C], F32, tag="zm2")
                nc.vector.tensor_scalar_min(zm, z[:], 0.0)
                em = pep.tile([128, C], F32, tag="em2")
                nc.scalar.activation(em, zm[:], mybir.ActivationFunctionType.Exp,
                                     bias=zero_b[:])
                zp = pep.tile([128, C], F32, tag="zp2")
                nc.vector.tensor_scalar_max(zp, z[:], 0.0)
                h3 = pep.tile([128, C], BF16, tag="h3")
                nc.vector.scalar_tensor_tensor(
                    out=h3, in0=em[:], scalar=-1.0, in1=zp[:],
                    op0=mybir.AluOpType.add, op1=mybir.AluOpType.add)
                psT = pps.tile([C, 128], BF16, tag="psmisc2")
                nc.tensor.transpose(psT, h3[:], ident_sb[:])
                h3T = pep.tile([C, 128], BF16, tag="h3T")
                nc.scalar.copy(h3T, psT[:])
                psL = pps.tile([128, cfg.classes], F32, tag="psmisc2")
                nc.tensor.matmul(out=psL, lhsT=h3T[:], rhs=Wout_sb[:], start=True, stop=True)
                z3 = pep.tile([128, cfg.classes], F32, tag="z3")
                nc.vector.tensor_tensor(out=z3, in0=psL[:], in1=bout_sb[:],
                                        op=mybir.AluOpType.add)
                mx = pep.tile([128, 1], F32, tag="mx")
                nc.vector.tensor_reduce(mx, z3[:], axis=mybir.AxisListType.X,
                                        op=mybir.AluOpType.max)
                nc.vector.tensor_scalar(out=zs_buf[:, t, :], in0=z3[:], scalar1=mx[:],
                                        scalar2=None, op0=mybir.AluOpType.subtract)
                es3 = pep.tile([128, cfg.classes], F32, tag="es3")
                nc.scalar.activation(es3, zs_buf[:, t, :],
                                     mybir.ActivationFunctionType.Exp,
                                     bias=zero_b[:], accum_out=ss_buf[:, t:t + 1])

            edge_phase(tab2, row2, C, 1, a_d2_sb, mneg2_b, post2)

            # ---- batched log-softmax tail: one Ln + one subtract + one DMA ----
            with tc.tile_pool(name="tail", bufs=1) as pt:
                lg = pt.tile([128, T], F32)
                nc.scalar.activation(lg, ss_buf[:],
                                     mybir.ActivationFunctionType.Ln,
                                     bias=zero_b[:])
                oo = pt.tile([128, T, cfg.classes], F32)
                nc.vector.tensor_tensor(
                    out=oo, in0=zs_buf[:],
                    in1=lg[:].unsqueeze(2).broadcast_to([128, T, cfg.classes]),
                    op=mybir.AluOpType.subtract)
                nc.sync.dma_start(
                    out=bass.AP(tensor=out.ap().tensor, offset=0,
                                ap=[[cfg.classes, 128],
                                    [128 * cfg.classes, T],
                                    [1, cfg.classes]]),
                    in_=oo[:])

    nc.compile()
    return nc


# ============================ public entry point ============================

_CACHE = {}


def kernel(**inputs):
    import numpy as np
    cfg = Cfg()
    hd = prep_host_data(cfg, inputs)
    key = ("gat_v2", cfg.n_pad, cfg.total_chunks, cfg.total_idx16,
           tuple(cfg.c_tb.reshape(-1).tolist()))
    nc = _CACHE.get(key)
    if nc is None:
        nc = build_program(cfg)
        _CACHE.clear()
        _CACHE[key] = nc
    from concourse.bass_utils import run_bass_kernel_spmd
    res = None
    last_err = None
    for _attempt in range(3):
        try:
            res = run_bass_kernel_spmd(nc, hd.per_core, list(range(cfg.n_cores)))
            break
        except Exception as e:  # device may need one reconnect after a crash
            last_err = e
    if res is None:
        raise last_err
    full = np.concatenate([res.results[i]["out"] for i in range(cfg.n_cores)],
                          axis=0)
    out = full[hd.perm[:cfg.n_nodes]]
    return np.ascontiguousarray(out.astype(np.float32))



# revision 34
# speedup vs baseline: 1.0741x; 1.0741x over previous
"""Self-contained TRN2 Bass kernel for the 2-layer multi-head GAT problem.

kernel(**inputs) -> np.ndarray [100000, 40] float32 (log_softmax outputs).

Strategy (v4): dst-sharded edge parallelism across 8 NeuronCores.

Layer 1 dense phase is REPLICATED: every core computes the full node table
(msg | a_s) for all 102400 padded nodes (x and W1 are replicated inputs), so
there is NO AllGather before the first edge phase — block-b gathers start as
soon as block b of the local table is written. a_d1 = x @ (W1 @ att_d) is
linear in x, so the host precomputes it per edge slot as a small bf16
constant stream (no device broadcast needed at all).

Layer 2's table depends on the (sharded) layer-1 aggregation, so it keeps a
single AllGather; its a_d2[dst] values are fetched by a LOCAL dst-indexed
dma_gather from the core's own shard (256B rows).

Edge phases gather source rows with dma_gather (int16 idx; one call per
(group, block) spanning the group's G tiles — middle pads point at row 0 and
are counted, tail pads are -1 — enabled by a larger SWDGE descriptor ring).
Per-group constants (src idx, a_d1 stream, ad2 idx) are staged from DRAM in
one DMA per group. Aggregation per 128-dst tile is one-hot matmuls in PSUM
with the segment softmax folded into a single normalize at the end
(exp-shift constant M is softmax-invariant).
"""

import math
import sys
from contextlib import ExitStack
from dataclasses import dataclass, field

import numpy as np

sys.path.insert(0, "/opt/trn_rl_repo")

import concourse.bacc as bacc
import concourse.bass as bass
import concourse.tile as tile
from concourse import mybir
from concourse.masks import make_identity

F32 = mybir.dt.float32
BF16 = mybir.dt.bfloat16
I16 = mybir.dt.int16


@dataclass
class Cfg:
    n_nodes: int = 100000
    f_in: int = 128
    hid: int = 32
    heads: int = 8
    classes: int = 40
    n_cores: int = 8
    tiles_per_core: int = 100
    n_blocks: int = 4     # int16 gather blocks (25600 rows each)
    group: int = 2        # tiles per gather-call group
    m1: float = 16.0      # exp-shift layer 1
    m2: float = 16.0      # exp-shift layer 2
    neg_slope: float = 0.2
    ablate: str = ""
    repeat: int = 1
    c_tb: object = None    # [T, B] chunks per (tile, block)
    # derived host bookkeeping (set in prep_host_data)
    grp_chunks: object = None   # [NG] chunks per group
    grp_choff: object = None    # [NG] chunk offset of group in global stream
    call_caps: object = None    # [NG, B] capacity (rows) per call
    call_ioff: object = None    # [NG, B] idx16 col offset per call (global)
    call_coff: object = None    # [NG, B] chunk offset of call within group
    tile_runs: object = None    # [T] list of (chunk_pos_in_group, count) runs
    grp_ioff: object = None     # [NG] combined-stream col offset of group
    grp_icols: object = None    # [NG] src idx16 cols of group
    total_ccols: int = 0        # combined per-group constant stream cols
    total_chunks: int = 0

    @property
    def d1(self):
        return self.heads * self.hid

    @property
    def n_pad(self):
        return self.n_cores * self.tiles_per_core * 128

    @property
    def nodes_per_core(self):
        return self.tiles_per_core * 128

    @property
    def block_rows(self):
        assert self.n_pad % self.n_blocks == 0
        return self.n_pad // self.n_blocks  # 25600 (< int16 max)

    @property
    def tiles_per_block(self):
        return self.block_rows // 128

    @property
    def n_groups(self):
        assert self.tiles_per_core % self.group == 0
        return self.tiles_per_core // self.group

    @property
    def row1(self):
        # bf16 cols of table1 row (msg 256 | a_s 8 | junk); 256B-mult stride
        need = self.d1 + self.heads
        return ((need + 127) // 128) * 128

    @property
    def row1_w(self):
        # written cols of a table1 row (rest is never read)
        return self.d1 + self.heads

    @property
    def row2(self):
        need = self.hid + 2  # h2 | a_s2 | a_d2
        return ((need + 127) // 128) * 128

    @property
    def cbg(self):
        # max chunks per group (tile alloc size)
        return int(self.grp_chunks.max())


def degree_balance_perm(dst: np.ndarray, cfg: Cfg) -> np.ndarray:
    """pi[old_id] = new_id; in-degrees balanced across 128-node tiles via
    snake round-robin over tiles in descending-degree order. Vectorized."""
    n, npad = cfg.n_nodes, cfg.n_pad
    deg = np.bincount(dst, minlength=n).astype(np.int64) + 1
    order = np.argsort(-deg, kind="stable")
    n_tiles = npad // 128
    i = np.arange(n, dtype=np.int64)
    rnd = i // n_tiles
    pos = i % n_tiles
    tl = np.where(rnd % 2 == 0, pos, n_tiles - 1 - pos)
    slot = rnd
    assert slot.max() < 128
    pi = np.empty(npad, dtype=np.int64)
    pi[order] = tl * 128 + slot
    # pad ids -> remaining slots
    used = np.zeros(npad, dtype=bool)
    used[pi[:n]] = True
    pi[n:] = np.flatnonzero(~used)
    return pi


@dataclass
class HostData:
    perm: np.ndarray
    inv_perm: np.ndarray
    per_core: list  # dict of input arrays per core


def wrap16(idx_1d: np.ndarray) -> np.ndarray:
    """[n] -> [16, n/16] wrapped (j at [j%16, j//16]), tiled to [128, n/16]."""
    n = idx_1d.shape[0]
    assert n % 16 == 0
    w = idx_1d.reshape(n // 16, 16).T.copy()  # [16, n/16]
    return np.tile(w, (8, 1))  # [128, n/16]


def interleave_cols(H, C):
    """perm p: new col j=w*H+h takes old col h*C+w. Returns old-index array
    such that new[:, j] = old[:, p[j]]."""
    p = np.empty(H * C, dtype=np.int64)
    for w in range(C):
        for h in range(H):
            p[w * H + h] = h * C + w
    return p


def prep_host_data(cfg: Cfg, inputs: dict) -> HostData:
    import ml_dtypes
    n, npad = cfg.n_nodes, cfg.n_pad
    H, C = cfg.heads, cfg.hid
    T, B, G = cfg.tiles_per_core, cfg.n_blocks, cfg.group
    NG = cfg.n_groups
    npc = cfg.nodes_per_core
    BR = cfg.block_rows
    x = np.asarray(inputs["x"], dtype=np.float32)
    ei = np.asarray(inputs["edge_index"])
    src0 = ei[0].astype(np.int64)
    dst0 = ei[1].astype(np.int64)
    loops = np.arange(n, dtype=np.int64)
    src0 = np.concatenate([src0, loops])
    dst0 = np.concatenate([dst0, loops])

    perm = degree_balance_perm(dst0, cfg)
    inv_perm = np.argsort(perm)
    src = perm[src0]
    dst = perm[dst0]

    # --- weights ---
    W1 = np.asarray(inputs["W1"], dtype=np.float64)      # [F, H*C]
    att_s1 = np.asarray(inputs["att_s1"], dtype=np.float64)  # [H, C]
    att_d1 = np.asarray(inputs["att_d1"], dtype=np.float64)
    b1 = np.asarray(inputs["b1"], dtype=np.float32)
    W2 = np.asarray(inputs["W2"], dtype=np.float64)      # [H*C, C]
    att_s2 = np.asarray(inputs["att_s2"], dtype=np.float64)  # [1, C]
    att_d2 = np.asarray(inputs["att_d2"], dtype=np.float64)
    b2 = np.asarray(inputs["b2"], dtype=np.float32)
    Wout = np.asarray(inputs["Wout"], dtype=np.float32)
    bout = np.asarray(inputs["bout"], dtype=np.float32)

    pcols = interleave_cols(H, C)  # head-minor msg column order
    Ws1 = np.zeros((cfg.f_in, H), dtype=np.float64)
    Wd1 = np.zeros((cfg.f_in, H), dtype=np.float64)
    for h in range(H):
        Ws1[:, h] = W1[:, h * C:(h + 1) * C] @ att_s1[h]
        Wd1[:, h] = W1[:, h * C:(h + 1) * C] @ att_d1[h]
    W1msg = W1[:, pcols]  # [F, 256] head-minor
    W1ext = np.concatenate([W1msg, Ws1], axis=1).astype(np.float32)
    b1p = b1[pcols]
    # W2ext rows follow the head-minor order of layer-1 msg cols
    Ws2 = W2 @ att_s2[0]
    Wd2 = W2 @ att_d2[0]
    W2ext = np.concatenate([W2, Ws2[:, None], Wd2[:, None]], axis=1)[pcols].astype(np.float32)

    # padded node features + host-computed a_d1 (linear in x)
    x_pad = np.zeros((npad, cfg.f_in), dtype=np.float32)
    x_pad[perm[:n]] = x
    xT = x_pad.T.astype(ml_dtypes.bfloat16)  # [F, npad]
    ad1_all = (x_pad @ Wd1.astype(np.float32))  # [npad, H] f32

    # --- src node -> (block, idx16) mapping (global 25600-row blocks) ---
    sblock = src // BR
    sloc_all = (src % BR).astype(np.int16)

    # --- per-core edge prep ---
    core_of_edge = dst // npc
    per_core = []
    for k in range(cfg.n_cores):
        m = core_of_edge == k
        es_, ed = src[m], dst[m]
        eb, el = sblock[m], sloc_all[m]
        tile_of = (ed - k * npc) // 128
        key = tile_of * B + eb
        order = np.argsort(key, kind="stable")
        es_, ed, el = es_[order], ed[order], el[order]
        counts = np.bincount(key, minlength=T * B)
        per_core.append(dict(es=es_, ed=ed, el=el, counts=counts, k=k))

    all_counts = np.stack([pc["counts"] for pc in per_core])  # [cores, T*B]
    c_tb = ((all_counts.max(axis=0) + 127) // 128).astype(np.int64)
    c_tb = np.maximum(c_tb, 1).reshape(T, B)
    cfg.c_tb = c_tb

    # --- group/call bookkeeping ---
    # per group gi: chunk stream layout = [b0: t0..tG-1 | b1: t0.. | ...]
    # one gather call per (gi, b) spans the group's G tiles; middle pads
    # (non-last tiles) point at row 0 and are counted, tail pads are -1.
    grp_chunks = np.zeros(NG, dtype=np.int64)
    call_caps = np.zeros((NG, B), dtype=np.int64)   # rows per call (padded)
    call_coff = np.zeros((NG, B), dtype=np.int64)   # chunk offset in group
    for gi in range(NG):
        off = 0
        for b in range(B):
            call_coff[gi, b] = off
            cap = 0
            for ti in range(G):
                cap += int(c_tb[gi * G + ti, b])
            call_caps[gi, b] = cap * 128
            off += cap
        grp_chunks[gi] = off
    grp_choff = np.concatenate([[0], np.cumsum(grp_chunks)])[:-1]
    total_chunks = int(grp_chunks.sum())

    # combined per-group constant stream: [src idx16 | ad2 idx16]
    grp_icols = np.zeros(NG, dtype=np.int64)
    for gi in range(NG):
        grp_icols[gi] = int(call_caps[gi].sum()) // 16
    grp_ccols = grp_icols + 8 * grp_chunks  # + ad2 idx16 (8c)
    grp_ioff = np.concatenate([[0], np.cumsum(grp_ccols)])[:-1]
    call_ioff = np.zeros((NG, G, B), dtype=np.int64)  # per (gi, ti, b)
    for gi in range(NG):
        acc = int(grp_ioff[gi])
        for b in range(B):
            for ti in range(G):
                call_ioff[gi, ti, b] = acc
                acc += int(c_tb[gi * G + ti, b]) * 8
    total_ccols = int(grp_ccols.sum())

    # per-tile chunk runs within its group (for aggregation)
    tile_runs = []
    for t in range(T):
        gi, ti = t // G, t % G
        runs = []
        for b in range(B):
            pos = int(call_coff[gi, b])
            for tj in range(ti):
                pos += int(c_tb[gi * G + tj, b])
            runs.append((pos, int(c_tb[t, b])))
        tile_runs.append(runs)

    cfg.grp_chunks = grp_chunks
    cfg.grp_choff = grp_choff
    cfg.call_caps = call_caps
    cfg.call_ioff = call_ioff
    cfg.call_coff = call_coff
    cfg.tile_runs = tile_runs
    cfg.grp_ioff = grp_ioff
    cfg.grp_icols = grp_icols
    cfg.total_ccols = total_ccols
    cfg.total_chunks = total_chunks

    # --- per-core streams ---
    per_core_arrays = []
    for pc in per_core:
        es_, ed, el, counts, k = pc["es"], pc["ed"], pc["el"], pc["counts"], pc["k"]
        starts = np.concatenate([[0], np.cumsum(counts)])
        cstream = np.zeros((128, total_ccols), dtype=np.int16)
        ad1s = np.zeros((128, total_chunks * H), dtype=ml_dtypes.bfloat16)
        dstloc = np.full(total_chunks * 128, -1.0, dtype=np.float32)
        ad1_flat = np.zeros((total_chunks * 128, H), dtype=np.float32)
        ad2_flat = np.zeros(total_chunks * 128, dtype=np.int16)
        ncnt = np.zeros(NG * G * B + NG * 8, dtype=np.int32)
        for gi in range(NG):
            gchoff = int(grp_choff[gi])
            for b in range(B):
                coff = int(call_coff[gi, b])
                fill = 0
                for ti in range(G):
                    t = gi * G + ti
                    gidx = t * B + b
                    s0, s1 = starts[gidx], starts[gidx + 1]
                    cnt = int(s1 - s0)
                    cap = int(c_tb[t, b]) * 128
                    assert cnt <= cap
                    idx_flat = np.full(cap, -1, dtype=np.int16)
                    idx_flat[:cnt] = el[s0:s1]
                    ncnt[(gi * G + ti) * B + b] = cnt
                    ib = int(call_ioff[gi, ti, b])
                    cstream[:, ib: ib + cap // 16] = wrap16(idx_flat)
                    # dstloc + a_d streams for this (t, b) run
                    sbase = (gchoff + coff) * 128 + fill
                    dl = (ed[s0:s1] - k * npc - t * 128).astype(np.float32)
                    dstloc[sbase: sbase + cnt] = dl
                    ad1_flat[sbase: sbase + cnt] = ad1_all[ed[s0:s1]]
                    ad2_flat[sbase: sbase + cnt] = (ed[s0:s1] - k * npc).astype(np.int16)
                    fill += cap
            # ad2 sub-call counts (<=8 chunks per call)
            nchk = int(grp_chunks[gi])
            for j, p0 in enumerate(range(0, nchk, 8)):
                ncnt[NG * G * B + gi * 8 + j] = min(8, nchk - p0) * 128
            # ad1 values + ad2 idx16, wrapped per group
            nchk = int(grp_chunks[gi])
            sbase = gchoff * 128
            a1 = ad1_flat[sbase: sbase + nchk * 128].astype(ml_dtypes.bfloat16)
            a1w = np.ascontiguousarray(
                a1.reshape(nchk, 128, H).transpose(1, 0, 2))
            ad1s[:, gchoff * H: (gchoff + nchk) * H] = a1w.reshape(128, nchk * H)
            a2w = wrap16(ad2_flat[sbase: sbase + nchk * 128])
            ib = int(grp_ioff[gi] + grp_icols[gi])
            cstream[:, ib: ib + 8 * nchk] = a2w
        dl3 = dstloc.reshape(total_chunks, 128)
        dstloc_part = np.ascontiguousarray(dl3.T).astype(ml_dtypes.bfloat16)
        arrs = dict(
            cstream=cstream,
            ad1s=ad1s,
            dstloc_p=dstloc_part,
            ncnt=ncnt.reshape(1, -1),
        )
        per_core_arrays.append(arrs)

    cbg = cfg.cbg
    iota_rep = np.zeros((1, 128 * cbg), dtype=np.float32)
    iota_rep[0] = np.repeat(np.arange(128, dtype=np.float32), cbg)
    iota_rep = iota_rep.astype(ml_dtypes.bfloat16)

    ck = min(128, cfg.d1)
    nk = cfg.d1 // ck
    W2chunk = np.ascontiguousarray(
        W2ext.reshape(nk, ck, C + 2).transpose(1, 0, 2)).reshape(ck, -1)
    for k, arrs in enumerate(per_core_arrays):
        arrs["xT"] = xT
        arrs["W1ext"] = W1ext.astype(ml_dtypes.bfloat16)
        arrs["W2ext"] = W2chunk.astype(ml_dtypes.bfloat16)
        arrs["Wout"] = Wout.astype(ml_dtypes.bfloat16)
        arrs["b1"] = b1p[None, :].astype(np.float32)
        arrs["b2"] = b2[None, :].astype(np.float32)
        arrs["bout"] = bout[None, :].astype(np.float32)
        arrs["iota_rep"] = iota_rep

    return HostData(perm=perm, inv_perm=inv_perm, per_core=per_core_arrays)


# ============================== device program ==============================

def build_program(cfg: Cfg, debug: bool = False):
    nc = bacc.Bacc("TRN2", target_bir_lowering=False, debug=debug,
                   num_devices=cfg.n_cores, num_swdge_queues=4,
                   dynamic_dma_scratch_size=16384)
    T, B, H, C, G = cfg.tiles_per_core, cfg.n_blocks, cfg.heads, cfg.hid, cfg.group
    NG = cfg.n_groups
    D1 = cfg.d1
    npc, npad = cfg.nodes_per_core, cfg.n_pad
    BR = cfg.block_rows
    TPB = cfg.tiles_per_block
    cbg = cfg.cbg
    row1, row1w, row2 = cfg.row1, cfg.row1_w, cfg.row2
    F = cfg.f_in
    groups8 = [list(range(cfg.n_cores))]
    ccols_max = int((cfg.grp_icols + 16 * cfg.grp_chunks).max())

    # ---- inputs ----
    xT = nc.dram_tensor("xT", [F, npad], BF16, kind="ExternalInput")
    W1ext = nc.dram_tensor("W1ext", [F, D1 + H], BF16, kind="ExternalInput")
    ck = min(128, D1)
    nk = D1 // ck
    W2ext = nc.dram_tensor("W2ext", [ck, nk * (C + 2)], BF16, kind="ExternalInput")
    Wout = nc.dram_tensor("Wout", [C, cfg.classes], BF16, kind="ExternalInput")
    b1 = nc.dram_tensor("b1", [1, D1], F32, kind="ExternalInput")
    b2 = nc.dram_tensor("b2", [1, C], F32, kind="ExternalInput")
    bout = nc.dram_tensor("bout", [1, cfg.classes], F32, kind="ExternalInput")
    iota_rep = nc.dram_tensor("iota_rep", [1, 128 * cbg], BF16, kind="ExternalInput")
    cstream = nc.dram_tensor("cstream", [128, cfg.total_ccols], I16, kind="ExternalInput")
    ad1s = nc.dram_tensor("ad1s", [128, cfg.total_chunks * H], BF16,
                          kind="ExternalInput")
    ncnt = nc.dram_tensor("ncnt", [1, NG * G * B + NG * 8], mybir.dt.int32,
                          kind="ExternalInput")
    dstloc_p = nc.dram_tensor("dstloc_p", [128, cfg.total_chunks], BF16, kind="ExternalInput")

    # ---- internal / output ----
    tab1 = [nc.dram_tensor(f"tab1_{b}", [BR, row1], BF16) for b in range(B)]
    tab2_sh = nc.dram_tensor("tab2_sh", [npc, row2], BF16)
    tab2 = nc.dram_tensor("tab2", [npad, row2], BF16, addr_space="Shared")
    out = nc.dram_tensor("out", [npc, cfg.classes], F32, kind="ExternalOutput")

    with tile.TileContext(nc, num_cores=cfg.n_cores) as tc, ExitStack() as ctx:
        consts = ctx.enter_context(tc.tile_pool(name="consts", bufs=1))

        # resident constants
        W1e_sb = consts.tile([F, D1 + H], BF16)
        nc.sync.dma_start(out=W1e_sb, in_=W1ext[:])
        W2e_sb = consts.tile([ck, nk, C + 2], BF16)
        nc.sync.dma_start(out=W2e_sb, in_=W2ext[:].rearrange("p (a c) -> p a c", a=nk))
        Wout_sb = consts.tile([C, cfg.classes], BF16)
        nc.sync.dma_start(out=Wout_sb, in_=Wout[:])
        b1_sb = consts.tile([128, D1], F32)
        nc.sync.dma_start(out=b1_sb, in_=bass.AP(
            tensor=b1.ap().tensor, offset=0, ap=[[0, 128], [1, D1]]))
        b2_sb = consts.tile([128, C], F32)
        nc.sync.dma_start(out=b2_sb, in_=bass.AP(
            tensor=b2.ap().tensor, offset=0, ap=[[0, 128], [1, C]]))
        bout_sb = consts.tile([128, cfg.classes], F32)
        nc.sync.dma_start(out=bout_sb, in_=bass.AP(
            tensor=bout.ap().tensor, offset=0, ap=[[0, 128], [1, cfg.classes]]))
        iota_sb = consts.tile([128, 128 * cbg], BF16)
        nc.sync.dma_start(out=iota_sb, in_=bass.AP(
            tensor=iota_rep.ap().tensor, offset=0, ap=[[0, 128], [1, 128 * cbg]]))
        ident_sb = consts.tile([128, 128], BF16)
        make_identity(nc, ident_sb)
        zero_b = consts.tile([128, 1], F32)
        nc.vector.memset(zero_b, 0.0)
        mneg1_b = consts.tile([128, 1], F32)
        nc.vector.memset(mneg1_b, -cfg.m1)
        mneg2_b = consts.tile([128, 1], F32)
        nc.vector.memset(mneg2_b, -cfg.m2)
        ncnt_sb = consts.tile([1, NG * G * B + NG * 8], mybir.dt.int32)
        nc.sync.dma_start(out=ncnt_sb, in_=ncnt[:])
        gcnt_regs = [nc.gpsimd.alloc_register(f"gcnt{i}")
                     for i in range(2 * G * B)]
        adcnt_regs = [nc.gpsimd.alloc_register(f"adcnt{i}") for i in range(16)]
        dlp_sb = consts.tile([128, cfg.total_chunks], BF16)
        nc.sync.dma_start(out=dlp_sb, in_=dstloc_p[:])
        zs_buf = consts.tile([128, T, cfg.classes], BF16)
        ss_buf = consts.tile([128, T], F32)
        nc.vector.memset(zs_buf, 0.0)
        nc.vector.memset(ss_buf, 1.0)

        for _rep in range(cfg.repeat):
            # ------- phase A: replicated dense; full table1, per block -------
            # rows are batched 8 tiles per DMA (one strided write) to keep
            # the HWDGE/SP queue off the critical path.
            RB = 8
            with tc.tile_pool(name="phA", bufs=2) as pa, \
                 tc.tile_pool(name="phA_x", bufs=2) as pax, \
                 tc.tile_pool(name="phA_ps", bufs=4, space="PSUM") as pap:
                for b in range(B):
                    xT_sb = pax.tile([F, BR], BF16, tag="xTc")
                    nc.sync.dma_start(out=xT_sb, in_=xT[:, b * BR:(b + 1) * BR])
                    for t0 in range(0, TPB, RB):
                        rows = pa.tile([128, RB, row1w], BF16, tag="rowA")
                        for j in range(RB):
                            t = t0 + j
                            ps = pap.tile([128, D1 + H], F32, tag="psA")
                            nc.tensor.matmul(out=ps,
                                             lhsT=xT_sb[:, t * 128:(t + 1) * 128],
                                             rhs=W1e_sb[:], start=True, stop=True)
                            if j % 2 == 0:
                                nc.scalar.copy(rows[:, j, :], ps[:])
                            else:
                                nc.vector.tensor_copy(rows[:, j, :], ps[:])
                        nc.sync.dma_start(
                            out=bass.AP(tensor=tab1[b].ap().tensor,
                                        offset=t0 * 128 * row1,
                                        ap=[[row1, 128], [128 * row1, RB],
                                            [1, row1w]]),
                            in_=rows[:])

            # ---------------- shared edge pipeline ----------------
            def edge_phase(block_ap, ad_src, rowN, DN, HN, adoff, mneg_bias,
                           post_tile):
                """block_ap(b) -> gather source AP for block b.
                ad_src: None (layer 1: a_d from cstream) or (tensor, row cols)
                for a local dst-indexed gather."""
                with tc.tile_pool(name="phB_g", bufs=2) as pgath, \
                     tc.tile_pool(name="phB_ga", bufs=2) as pgad, \
                     tc.tile_pool(name="phB_ix", bufs=2) as pidx, \
                     tc.tile_pool(name="phB_S", bufs=2) as pS, \
                     tc.tile_pool(name="phB_m", bufs=2) as pmsg, \
                     tc.tile_pool(name="phB_ps", bufs=2, space="PSUM") as pbp, \
                     tc.tile_pool(name="phB_ep", bufs=2) as pep, \
                     tc.tile_pool(name="phB_ps2", bufs=2, space="PSUM") as pbp2:
                    for gi in range(NG):
                        cbgi = int(cfg.grp_chunks[gi])
                        choff = int(cfg.grp_choff[gi])
                        gioff = int(cfg.grp_ioff[gi])
                        gicols = int(cfg.grp_icols[gi])
                        gccols = gicols + 8 * cbgi
                        cs = pidx.tile([128, ccols_max], I16, tag="cs")
                        nc.sync.dma_start(out=cs[:, 0:gccols],
                                          in_=cstream[:, gioff:gioff + gccols])
                        if ad_src is None:
                            a1t = pidx.tile([128, cbg * H], BF16, tag="a1t")
                            nc.sync.dma_start(
                                out=a1t[:, 0:cbgi * H],
                                in_=ad1s[:, choff * H:(choff + cbgi) * H])
                        g = pgath.tile([128, cbg, rowN], BF16, tag="gath")
                        if gi < 2:
                            nc.vector.memset(g, 0.0)
                        if ad_src is not None:
                            adtab, adrow = ad_src
                            gad = pgad.tile([128, cbg, adrow], BF16, tag="gad")
                            for j, p0 in enumerate(range(0, cbgi, 8)):
                                pc = min(8, cbgi - p0)
                                ci = NG * G * B + gi * 8 + j
                                areg = adcnt_regs[(gi % 2) * 8 + j]
                                nc.gpsimd.reg_load(areg, ncnt_sb[0:1, ci:ci + 1])
                                nc.gpsimd.dma_gather(
                                    gad[:, p0:p0 + pc, :], adtab[:],
                                    cs[:, gicols + p0 * 8: gicols + (p0 + pc) * 8],
                                    pc * 128, areg, adrow, elem_step=adrow,
                                    queue_num=j % nc.num_swdge_queues)
                            ad_ap = gad[:, 0:cbgi, adoff:adoff + HN]
                        else:
                            ad_ap = a1t[:, 0:cbgi * H].rearrange(
                                "p (c h) -> p c h", h=H)
                        for b in range(B):
                            coff = int(cfg.call_coff[gi, b])
                            for ti in range(G):
                                t = gi * G + ti
                                cap = int(cfg.c_tb[t, b]) * 128
                                i0 = int(cfg.call_ioff[gi, ti, b]) - gioff
                                ci = (gi * G + ti) * B + b
                                reg = gcnt_regs[(gi % 2) * G * B + ti * B + b]
                                nc.gpsimd.reg_load(reg, ncnt_sb[0:1, ci:ci + 1])
                                nc.gpsimd.dma_gather(
                                    g[:, coff:coff + cap // 128, :],
                                    block_ap(b),
                                    cs[:, i0:i0 + cap // 16],
                                    cap, reg, rowN, elem_step=rowN,
                                    queue_num=b % nc.num_swdge_queues)
                                coff += cap // 128
                        # one-hot S[e, d, c] (chunk innermost; 2x mode)
                        S = pS.tile([128, 128, cbg], BF16, tag="S")
                        nc.vector.tensor_tensor(
                            out=S[:, :, :cbgi],
                            in0=iota_sb[:].rearrange("p (q c) -> p q c", q=128)[:, :, :cbgi],
                            in1=dlp_sb[:, choff:choff + cbgi].unsqueeze(1).broadcast_to(
                                [128, 128, cbgi]),
                            op=mybir.AluOpType.is_equal)
                        # es = a_s[src] + a_d[dst] (+ leaky)
                        es = pep.tile([128, cbg, HN], F32, tag="es")
                        nc.vector.tensor_tensor(out=es[:, :cbgi, :],
                                                in0=ad_ap,
                                                in1=g[:, :cbgi, DN:DN + HN],
                                                op=mybir.AluOpType.add)
                        nc.vector.scalar_tensor_tensor(
                            out=es[:, :cbgi, :], in0=es[:, :cbgi, :],
                            scalar=cfg.neg_slope, in1=es[:, :cbgi, :],
                            op0=mybir.AluOpType.mult, op1=mybir.AluOpType.max)
                        # msgee = [g * ee (head-minor broadcast) | ee]
                        ee = pep.tile([128, cbg, HN], BF16, tag="ee")
                        nc.scalar.activation(ee[:, :cbgi, :], es[:, :cbgi, :],
                                             mybir.ActivationFunctionType.Exp,
                                             bias=mneg_bias[:], scale=1.0)
                        msgee = pmsg.tile([128, cbg, DN + HN], BF16, tag="msgee")
                        nc.vector.tensor_copy(msgee[:, :cbgi, DN:DN + HN],
                                              ee[:, :cbgi, :])
                        if HN > 1:
                            nc.vector.tensor_tensor(
                                out=msgee[:, :cbgi, 0:DN].rearrange(
                                    "p c (w h) -> p c w h", h=HN),
                                in0=g[:, :cbgi, 0:DN].rearrange(
                                    "p c (w h) -> p c w h", h=HN),
                                in1=ee[:, :cbgi, :].unsqueeze(2).broadcast_to(
                                    [128, cbgi, DN // HN, HN]),
                                op=mybir.AluOpType.mult)
                        else:
                            nc.vector.tensor_tensor(
                                out=msgee[:, :cbgi, 0:DN],
                                in0=g[:, :cbgi, 0:DN],
                                in1=ee[:, :cbgi, :].broadcast_to(
                                    [128, cbgi, DN]),
                                op=mybir.AluOpType.mult)
                        # aggregate per tile over its chunk runs
                        for ti in range(G):
                            t = gi * G + ti
                            runs = cfg.tile_runs[t]
                            chunks = [p for (pos, cnt) in runs
                                      for p in range(pos, pos + cnt)]
                            psG = pbp.tile([128, DN + HN], F32, tag="psG")
                            for jj, j in enumerate(chunks):
                                nc.tensor.matmul(out=psG, lhsT=S[:, :, j],
                                                 rhs=msgee[:, j, :],
                                                 start=(jj == 0),
                                                 stop=(jj == len(chunks) - 1))
                            post_tile(t, psG, pep, pbp2)

            # ---- layer-1 post-tile: divide, +b1, elu, h2 fold ----
            def post1(t, psG, pep, pps):
                den = pep.tile([128, H], F32, tag="den")
                nc.vector.tensor_scalar_add(den, psG[:, D1:D1 + H], 1e-30)
                rden = pep.tile([128, H], F32, tag="rden")
                nc.vector.reciprocal(rden, den[:])
                z = pep.tile([128, D1], F32, tag="z")
                nc.vector.tensor_tensor(
                    out=z[:].rearrange("p (w h) -> p w h", h=H),
                    in0=psG[:, 0:D1].rearrange("p (w h) -> p w h", h=H),
                    in1=rden[:].unsqueeze(1).broadcast_to([128, C, H]),
                    op=mybir.AluOpType.mult)
                nc.vector.tensor_tensor(out=z, in0=z[:], in1=b1_sb[:],
                                        op=mybir.AluOpType.add)
                zp = pep.tile([128, D1], F32, tag="zp")
                nc.vector.tensor_scalar_max(zp, z[:], 0.0)
                nc.vector.tensor_scalar_min(z, z[:], 0.0)  # in place
                em = pep.tile([128, D1], F32, tag="em")
                nc.scalar.activation(em, z[:], mybir.ActivationFunctionType.Exp,
                                     bias=zero_b[:])
                elu_bf = pep.tile([128, D1], BF16, tag="elubf")
                nc.vector.scalar_tensor_tensor(
                    out=elu_bf, in0=em[:], scalar=-1.0, in1=zp[:],
                    op0=mybir.AluOpType.add, op1=mybir.AluOpType.add)
                # h2 = eluT @ W2ext
                eluT = pep.tile([ck, nk, 128], BF16, tag="eluT")
                for kk in range(nk):
                    psT = pps.tile([ck, 128], BF16, tag="psmisc")
                    nc.tensor.transpose(psT, elu_bf[:, kk * ck:(kk + 1) * ck], ident_sb[:])
                    nc.scalar.copy(eluT[:, kk, :], psT[:])
                psH2 = pps.tile([128, C + 2], F32, tag="psmisc")
                for kk in range(nk):
                    nc.tensor.matmul(out=psH2, lhsT=eluT[:, kk, :], rhs=W2e_sb[:, kk, :],
                                     start=(kk == 0), stop=(kk == nk - 1))
                row = pep.tile([128, row2], BF16, tag="rowC")
                if row2 > C + 2:
                    nc.vector.memset(row[:, C + 2:], 0.0)
                nc.scalar.copy(row[:, 0:C + 2], psH2[:])
                nc.sync.dma_start(out=tab2_sh[t * 128:(t + 1) * 128, :], in_=row[:])

            if cfg.ablate == 'phaseA':
                with tc.tile_pool(name="dummy", bufs=1) as pd:
                    zt = pd.tile([128, T, cfg.classes], F32)
                    nc.vector.memset(zt, 0.0)
                    nc.sync.dma_start(
                        out=bass.AP(tensor=out.ap().tensor, offset=0,
                                    ap=[[cfg.classes, 128],
                                        [128 * cfg.classes, T],
                                        [1, cfg.classes]]),
                        in_=zt[:])
            if cfg.ablate != 'phaseA':
                edge_phase(lambda b: tab1[b][:], None, row1, D1, H, 0,
                           mneg1_b, post1)

            if cfg.ablate in ('phaseA', 'e1'):
                pass
            elif cfg.ablate == "nocoll":
                nc.sync.dma_start(out=tab2[0:npc, :], in_=tab2_sh[:])
            else:
                nc.gpsimd.collective_compute(
                    "AllGather", mybir.AluOpType.bypass, groups8,
                    ins=[tab2_sh[:]], outs=[tab2[:]])

            # ---- layer-2 post-tile: divide, +b2, elu, head, partial smax ----
            def post2(t, psG, pep, pps):
                den = pep.tile([128, 1], F32, tag="den2")
                nc.vector.tensor_scalar_add(den, psG[:, C:C + 1], 1e-30)
                rden = pep.tile([128, 1], F32, tag="rden2")
                nc.vector.reciprocal(rden, den[:])
                z = pep.tile([128, C], F32, tag="z2")
                nc.vector.tensor_scalar(out=z, in0=psG[:, 0:C], scalar1=rden[:],
                                        scalar2=None, op0=mybir.AluOpType.mult)
                nc.vector.tensor_tensor(out=z, in0=z[:], in1=b2_sb[:],
                                        op=mybir.AluOpType.add)
                zp = pep.tile([128, C], F32, tag="zp2")
                nc.vector.tensor_scalar_max(zp, z[:], 0.0)
                nc.vector.tensor_scalar_min(z, z[:], 0.0)  # in place
                em = pep.tile([128, C], F32, tag="em2")
                nc.scalar.activation(em, z[:], mybir.ActivationFunctionType.Exp,
                                     bias=zero_b[:])
                h3 = pep.tile([128, C], BF16, tag="h3")
                nc.vector.scalar_tensor_tensor(
                    out=h3, in0=em[:], scalar=-1.0, in1=zp[:],
                    op0=mybir.AluOpType.add, op1=mybir.AluOpType.add)
                psT = pps.tile([C, 128], BF16, tag="psmisc2")
                nc.tensor.transpose(psT, h3[:], ident_sb[:])
                h3T = pep.tile([C, 128], BF16, tag="h3T")
                nc.scalar.copy(h3T, psT[:])
                psL = pps.tile([128, cfg.classes], F32, tag="psmisc2")
                nc.tensor.matmul(out=psL, lhsT=h3T[:], rhs=Wout_sb[:], start=True, stop=True)
                z3 = pep.tile([128, cfg.classes], F32, tag="z3")
                nc.vector.tensor_tensor(out=z3, in0=psL[:], in1=bout_sb[:],
                                        op=mybir.AluOpType.add)
                mx = pep.tile([128, 1], F32, tag="mx")
                nc.vector.tensor_reduce(mx, z3[:], axis=mybir.AxisListType.X,
                                        op=mybir.AluOpType.max)
                nc.vector.tensor_scalar(out=zs_buf[:, t, :], in0=z3[:], scalar1=mx[:],
                                        scalar2=None, op0=mybir.AluOpType.subtract)
                es3 = pep.tile([128, cfg.classes], F32, tag="es3")
                nc.scalar.activation(es3, zs_buf[:, t, :],
                                     mybir.ActivationFunctionType.Exp,
                                     bias=zero_b[:], accum_out=ss_buf[:, t:t + 1])

            if cfg.ablate not in ('phaseA', 'e1'):
                edge_phase(lambda b: tab2[b * BR:(b + 1) * BR, :], (tab2_sh, row2),
                           row2, C, 1, C + 1, mneg2_b, post2)

            # ---- batched log-softmax tail: one Ln + one subtract + one DMA ----
            with tc.tile_pool(name="tail", bufs=1) as pt:
                if cfg.ablate == 'phaseA':
                    pt = pt  # pools still open/close
                
                lg = pt.tile([128, T], F32)
                nc.scalar.activation(lg, ss_buf[:],
                                     mybir.ActivationFunctionType.Ln,
                                     bias=zero_b[:])
                oo = pt.tile([128, T, cfg.classes], F32)
                nc.vector.tensor_tensor(
                    out=oo, in0=zs_buf[:],
                    in1=lg[:].unsqueeze(2).broadcast_to([128, T, cfg.classes]),
                    op=mybir.AluOpType.subtract)
                nc.sync.dma_start(
                    out=bass.AP(tensor=out.ap().tensor, offset=0,
                                ap=[[cfg.classes, 128],
                                    [128 * cfg.classes, T],
                                    [1, cfg.classes]]),
                    in_=oo[:])

    nc.compile()
    return nc


# ============================ public entry point ============================

_CACHE = {}


def kernel(**inputs):
    import numpy as np
    cfg = Cfg()
    hd = prep_host_data(cfg, inputs)
    key = ("gat_v4", cfg.n_pad, cfg.total_chunks, cfg.total_ccols,
           tuple(cfg.c_tb.reshape(-1).tolist()))
    nc = _CACHE.get(key)
    if nc is None:
        nc = build_program(cfg)
        _CACHE.clear()
        _CACHE[key] = nc
    from concourse.bass_utils import run_bass_kernel_spmd
    res = None
    last_err = None
    for _attempt in range(3):
        try:
            res = run_bass_kernel_spmd(nc, hd.per_core, list(range(cfg.n_cores)))
            break
        except Exception as e:  # device may need one reconnect after a crash
            last_err = e
    if res is None:
        raise last_err
    full = np.concatenate([res.results[i]["out"] for i in range(cfg.n_cores)],
                          axis=0)
    out = full[hd.perm[:cfg.n_nodes]]
    return np.ascontiguousarray(out.astype(np.float32))


# revision 35
# speedup vs baseline: 3.6594x; 3.4069x over previous
"""Self-contained TRN2 Bass kernel for the 2-layer multi-head GAT problem.

kernel(**inputs) -> np.ndarray [100000, 40] float32 (log_softmax outputs).

Strategy (v4): dst-sharded edge parallelism across 8 NeuronCores.

Layer 1 dense phase is REPLICATED: every core computes the full node table
(msg | a_s) for all 102400 padded nodes (x and W1 are replicated inputs), so
there is NO AllGather before the first edge phase — block-b gathers start as
soon as block b of the local table is written. a_d1 = x @ (W1 @ att_d) is
linear in x, so the host precomputes it per edge slot as a small bf16
constant stream (no device broadcast needed at all).

Layer 2's table depends on the (sharded) layer-1 aggregation, so it keeps a
single AllGather; its a_d2[dst] values are fetched by a LOCAL dst-indexed
dma_gather from the core's own shard (256B rows).

Edge phases gather source rows with dma_gather (int16 idx; one call per
(tile, block) — calls MUST stay <= 1024 rows: the SWDGE ucode assumes the
default 16KB descriptor scratch, so larger rings/calls crash the device).
Per-group constants (src idx, ad2 idx) are staged from DRAM in one DMA per
group. Aggregation per 128-dst tile is one-hot matmuls in PSUM with the
segment softmax folded into a single normalize at the end (exp-shift
constant M is softmax-invariant).
"""

import math
import sys
from contextlib import ExitStack
from dataclasses import dataclass, field

import numpy as np

sys.path.insert(0, "/opt/trn_rl_repo")

import concourse.bacc as bacc
import concourse.bass as bass
import concourse.tile as tile
from concourse import mybir
from concourse.masks import make_identity

F32 = mybir.dt.float32
BF16 = mybir.dt.bfloat16
I16 = mybir.dt.int16


@dataclass
class Cfg:
    n_nodes: int = 100000
    f_in: int = 128
    hid: int = 32
    heads: int = 8
    classes: int = 40
    n_cores: int = 8
    tiles_per_core: int = 100
    n_blocks: int = 4     # int16 gather blocks (25600 rows each)
    group: int = 2        # tiles per gather-call group
    m1: float = 16.0      # exp-shift layer 1
    m2: float = 16.0      # exp-shift layer 2
    neg_slope: float = 0.2
    ablate: str = ""
    repeat: int = 1
    c_tb: object = None    # [T, B] chunks per (tile, block)
    # derived host bookkeeping (set in prep_host_data)
    grp_chunks: object = None   # [NG] chunks per group
    grp_choff: object = None    # [NG] chunk offset of group in global stream
    call_caps: object = None    # [NG, B] capacity (rows) per call
    call_ioff: object = None    # [NG, B] idx16 col offset per call (global)
    call_coff: object = None    # [NG, B] chunk offset of call within group
    tile_runs: object = None    # [T] list of (chunk_pos_in_group, count) runs
    grp_ioff: object = None     # [NG] combined-stream col offset of group
    grp_icols: object = None    # [NG] src idx16 cols of group
    total_ccols: int = 0        # combined per-group constant stream cols
    total_chunks: int = 0

    @property
    def d1(self):
        return self.heads * self.hid

    @property
    def n_pad(self):
        return self.n_cores * self.tiles_per_core * 128

    @property
    def nodes_per_core(self):
        return self.tiles_per_core * 128

    @property
    def block_rows(self):
        assert self.n_pad % self.n_blocks == 0
        return self.n_pad // self.n_blocks  # 25600 (< int16 max)

    @property
    def tiles_per_block(self):
        return self.block_rows // 128

    @property
    def n_groups(self):
        assert self.tiles_per_core % self.group == 0
        return self.tiles_per_core // self.group

    @property
    def row1(self):
        # bf16 cols of table1 row (msg 256 | a_s 8 | junk); 256B-mult stride
        need = self.d1 + self.heads
        return ((need + 127) // 128) * 128

    @property
    def row1_w(self):
        # written cols of a table1 row (rest is never read)
        return self.d1 + self.heads

    @property
    def row2(self):
        need = self.hid + 2  # h2 | a_s2 | a_d2
        return ((need + 127) // 128) * 128

    @property
    def cbg(self):
        # max chunks per group (tile alloc size)
        return int(self.grp_chunks.max())


def degree_balance_perm(dst: np.ndarray, cfg: Cfg) -> np.ndarray:
    """pi[old_id] = new_id; in-degrees balanced across 128-node tiles via
    snake round-robin over tiles in descending-degree order. Vectorized."""
    n, npad = cfg.n_nodes, cfg.n_pad
    deg = np.bincount(dst, minlength=n).astype(np.int64) + 1
    order = np.argsort(-deg, kind="stable")
    n_tiles = npad // 128
    i = np.arange(n, dtype=np.int64)
    rnd = i // n_tiles
    pos = i % n_tiles
    tl = np.where(rnd % 2 == 0, pos, n_tiles - 1 - pos)
    slot = rnd
    assert slot.max() < 128
    pi = np.empty(npad, dtype=np.int64)
    pi[order] = tl * 128 + slot
    # pad ids -> remaining slots
    used = np.zeros(npad, dtype=bool)
    used[pi[:n]] = True
    pi[n:] = np.flatnonzero(~used)
    return pi


@dataclass
class HostData:
    perm: np.ndarray
    inv_perm: np.ndarray
    per_core: list  # dict of input arrays per core


def wrap16(idx_1d: np.ndarray) -> np.ndarray:
    """[n] -> [16, n/16] wrapped (j at [j%16, j//16]), tiled to [128, n/16]."""
    n = idx_1d.shape[0]
    assert n % 16 == 0
    w = idx_1d.reshape(n // 16, 16).T.copy()  # [16, n/16]
    return np.tile(w, (8, 1))  # [128, n/16]


def interleave_cols(H, C):
    """perm p: new col j=w*H+h takes old col h*C+w. Returns old-index array
    such that new[:, j] = old[:, p[j]]."""
    p = np.empty(H * C, dtype=np.int64)
    for w in range(C):
        for h in range(H):
            p[w * H + h] = h * C + w
    return p


def prep_host_data(cfg: Cfg, inputs: dict) -> HostData:
    import ml_dtypes
    n, npad = cfg.n_nodes, cfg.n_pad
    H, C = cfg.heads, cfg.hid
    T, B, G = cfg.tiles_per_core, cfg.n_blocks, cfg.group
    NG = cfg.n_groups
    npc = cfg.nodes_per_core
    BR = cfg.block_rows
    x = np.asarray(inputs["x"], dtype=np.float32)
    ei = np.asarray(inputs["edge_index"])
    src0 = ei[0].astype(np.int64)
    dst0 = ei[1].astype(np.int64)
    loops = np.arange(n, dtype=np.int64)
    src0 = np.concatenate([src0, loops])
    dst0 = np.concatenate([dst0, loops])

    perm = degree_balance_perm(dst0, cfg)
    inv_perm = np.argsort(perm)
    src = perm[src0]
    dst = perm[dst0]

    # --- weights ---
    W1 = np.asarray(inputs["W1"], dtype=np.float64)      # [F, H*C]
    att_s1 = np.asarray(inputs["att_s1"], dtype=np.float64)  # [H, C]
    att_d1 = np.asarray(inputs["att_d1"], dtype=np.float64)
    b1 = np.asarray(inputs["b1"], dtype=np.float32)
    W2 = np.asarray(inputs["W2"], dtype=np.float64)      # [H*C, C]
    att_s2 = np.asarray(inputs["att_s2"], dtype=np.float64)  # [1, C]
    att_d2 = np.asarray(inputs["att_d2"], dtype=np.float64)
    b2 = np.asarray(inputs["b2"], dtype=np.float32)
    Wout = np.asarray(inputs["Wout"], dtype=np.float32)
    bout = np.asarray(inputs["bout"], dtype=np.float32)

    pcols = interleave_cols(H, C)  # head-minor msg column order
    Ws1 = np.zeros((cfg.f_in, H), dtype=np.float64)
    Wd1 = np.zeros((cfg.f_in, H), dtype=np.float64)
    for h in range(H):
        Ws1[:, h] = W1[:, h * C:(h + 1) * C] @ att_s1[h]
        Wd1[:, h] = W1[:, h * C:(h + 1) * C] @ att_d1[h]
    W1msg = W1[:, pcols]  # [F, 256] head-minor
    W1ext = np.concatenate([W1msg, Ws1], axis=1).astype(np.float32)
    b1p = b1[pcols]
    # W2ext rows follow the head-minor order of layer-1 msg cols
    Ws2 = W2 @ att_s2[0]
    Wd2 = W2 @ att_d2[0]
    W2ext = np.concatenate([W2, Ws2[:, None], Wd2[:, None]], axis=1)[pcols].astype(np.float32)

    # padded node features + host-computed a_d1 (linear in x)
    x_pad = np.zeros((npad, cfg.f_in), dtype=np.float32)
    x_pad[perm[:n]] = x
    xT = x_pad.T.astype(ml_dtypes.bfloat16)  # [F, npad]
    ad1_all = (x_pad @ Wd1.astype(np.float32))  # [npad, H] f32

    # --- src node -> (block, idx16) mapping (global 25600-row blocks) ---
    sblock = src // BR
    sloc_all = (src % BR).astype(np.int16)

    # --- per-core edge prep ---
    core_of_edge = dst // npc
    per_core = []
    for k in range(cfg.n_cores):
        m = core_of_edge == k
        es_, ed = src[m], dst[m]
        eb, el = sblock[m], sloc_all[m]
        tile_of = (ed - k * npc) // 128
        key = tile_of * B + eb
        order = np.argsort(key, kind="stable")
        es_, ed, el = es_[order], ed[order], el[order]
        counts = np.bincount(key, minlength=T * B)
        per_core.append(dict(es=es_, ed=ed, el=el, counts=counts, k=k))

    all_counts = np.stack([pc["counts"] for pc in per_core])  # [cores, T*B]
    c_tb = ((all_counts.max(axis=0) + 127) // 128).astype(np.int64)
    c_tb = np.maximum(c_tb, 1).reshape(T, B)
    cfg.c_tb = c_tb

    # --- group/call bookkeeping ---
    # per group gi: chunk stream layout = [b0: t0..tG-1 | b1: t0.. | ...]
    # one gather call per (gi, b) spans the group's G tiles; middle pads
    # (non-last tiles) point at row 0 and are counted, tail pads are -1.
    grp_chunks = np.zeros(NG, dtype=np.int64)
    call_caps = np.zeros((NG, B), dtype=np.int64)   # rows per call (padded)
    call_coff = np.zeros((NG, B), dtype=np.int64)   # chunk offset in group
    for gi in range(NG):
        off = 0
        for b in range(B):
            call_coff[gi, b] = off
            cap = 0
            for ti in range(G):
                cap += int(c_tb[gi * G + ti, b])
            call_caps[gi, b] = cap * 128
            off += cap
        grp_chunks[gi] = off
    grp_choff = np.concatenate([[0], np.cumsum(grp_chunks)])[:-1]
    total_chunks = int(grp_chunks.sum())

    # combined per-group constant stream: [src idx16 | ad2 idx16]
    grp_icols = np.zeros(NG, dtype=np.int64)
    for gi in range(NG):
        grp_icols[gi] = int(call_caps[gi].sum()) // 16
    grp_ccols = grp_icols + 8 * grp_chunks  # + ad2 idx16 (8c)
    grp_ioff = np.concatenate([[0], np.cumsum(grp_ccols)])[:-1]
    call_ioff = np.zeros((NG, G, B), dtype=np.int64)  # per (gi, ti, b)
    for gi in range(NG):
        acc = int(grp_ioff[gi])
        for b in range(B):
            for ti in range(G):
                call_ioff[gi, ti, b] = acc
                acc += int(c_tb[gi * G + ti, b]) * 8
    total_ccols = int(grp_ccols.sum())

    # per-tile chunk runs within its group (for aggregation)
    tile_runs = []
    for t in range(T):
        gi, ti = t // G, t % G
        runs = []
        for b in range(B):
            pos = int(call_coff[gi, b])
            for tj in range(ti):
                pos += int(c_tb[gi * G + tj, b])
            runs.append((pos, int(c_tb[t, b])))
        tile_runs.append(runs)

    cfg.grp_chunks = grp_chunks
    cfg.grp_choff = grp_choff
    cfg.call_caps = call_caps
    cfg.call_ioff = call_ioff
    cfg.call_coff = call_coff
    cfg.tile_runs = tile_runs
    cfg.grp_ioff = grp_ioff
    cfg.grp_icols = grp_icols
    cfg.total_ccols = total_ccols
    cfg.total_chunks = total_chunks

    # --- per-core streams ---
    per_core_arrays = []
    for pc in per_core:
        es_, ed, el, counts, k = pc["es"], pc["ed"], pc["el"], pc["counts"], pc["k"]
        starts = np.concatenate([[0], np.cumsum(counts)])
        cstream = np.zeros((128, total_ccols), dtype=np.int16)
        ad1s = np.zeros((128, total_chunks * H), dtype=ml_dtypes.bfloat16)
        dstloc = np.full(total_chunks * 128, -1.0, dtype=np.float32)
        ad1_flat = np.zeros((total_chunks * 128, H), dtype=np.float32)
        ad2_flat = np.zeros(total_chunks * 128, dtype=np.int16)
        ncnt = np.zeros(NG * G * B + NG * 8, dtype=np.int32)
        for gi in range(NG):
            gchoff = int(grp_choff[gi])
            for b in range(B):
                coff = int(call_coff[gi, b])
                fill = 0
                for ti in range(G):
                    t = gi * G + ti
                    gidx = t * B + b
                    s0, s1 = starts[gidx], starts[gidx + 1]
                    cnt = int(s1 - s0)
                    cap = int(c_tb[t, b]) * 128
                    assert cnt <= cap
                    idx_flat = np.full(cap, -1, dtype=np.int16)
                    idx_flat[:cnt] = el[s0:s1]
                    ncnt[(gi * G + ti) * B + b] = cnt
                    ib = int(call_ioff[gi, ti, b])
                    cstream[:, ib: ib + cap // 16] = wrap16(idx_flat)
                    # dstloc + a_d streams for this (t, b) run
                    sbase = (gchoff + coff) * 128 + fill
                    dl = (ed[s0:s1] - k * npc - t * 128).astype(np.float32)
                    dstloc[sbase: sbase + cnt] = dl
                    ad1_flat[sbase: sbase + cnt] = ad1_all[ed[s0:s1]]
                    ad2_flat[sbase: sbase + cnt] = (ed[s0:s1] - k * npc).astype(np.int16)
                    fill += cap
            # ad2 sub-call counts (<=8 chunks per call)
            nchk = int(grp_chunks[gi])
            for j, p0 in enumerate(range(0, nchk, 8)):
                ncnt[NG * G * B + gi * 8 + j] = min(8, nchk - p0) * 128
            # ad1 values + ad2 idx16, wrapped per group
            nchk = int(grp_chunks[gi])
            sbase = gchoff * 128
            a1 = ad1_flat[sbase: sbase + nchk * 128].astype(ml_dtypes.bfloat16)
            a1w = np.ascontiguousarray(
                a1.reshape(nchk, 128, H).transpose(1, 0, 2))
            ad1s[:, gchoff * H: (gchoff + nchk) * H] = a1w.reshape(128, nchk * H)
            a2w = wrap16(ad2_flat[sbase: sbase + nchk * 128])
            ib = int(grp_ioff[gi] + grp_icols[gi])
            cstream[:, ib: ib + 8 * nchk] = a2w
        dl3 = dstloc.reshape(total_chunks, 128)
        dstloc_part = np.ascontiguousarray(dl3.T).astype(ml_dtypes.bfloat16)
        arrs = dict(
            cstream=cstream,
            ad1s=ad1s,
            dstloc_p=dstloc_part,
            ncnt=ncnt.reshape(1, -1),
        )
        per_core_arrays.append(arrs)

    cbg = cfg.cbg
    iota_rep = np.zeros((1, 128 * cbg), dtype=np.float32)
    iota_rep[0] = np.repeat(np.arange(128, dtype=np.float32), cbg)
    iota_rep = iota_rep.astype(ml_dtypes.bfloat16)

    ck = min(128, cfg.d1)
    nk = cfg.d1 // ck
    W2chunk = np.ascontiguousarray(
        W2ext.reshape(nk, ck, C + 2).transpose(1, 0, 2)).reshape(ck, -1)
    for k, arrs in enumerate(per_core_arrays):
        arrs["xT"] = xT
        arrs["W1ext"] = W1ext.astype(ml_dtypes.bfloat16)
        arrs["W2ext"] = W2chunk.astype(ml_dtypes.bfloat16)
        arrs["Wout"] = Wout.astype(ml_dtypes.bfloat16)
        arrs["b1"] = b1p[None, :].astype(np.float32)
        arrs["b2"] = b2[None, :].astype(np.float32)
        arrs["bout"] = bout[None, :].astype(np.float32)
        arrs["iota_rep"] = iota_rep

    return HostData(perm=perm, inv_perm=inv_perm, per_core=per_core_arrays)


# ============================== device program ==============================

def build_program(cfg: Cfg, debug: bool = False):
    nc = bacc.Bacc("TRN2", target_bir_lowering=False, debug=debug,
                   num_devices=cfg.n_cores, num_swdge_queues=4,
                   dynamic_dma_scratch_size=16384)
    T, B, H, C, G = cfg.tiles_per_core, cfg.n_blocks, cfg.heads, cfg.hid, cfg.group
    NG = cfg.n_groups
    D1 = cfg.d1
    npc, npad = cfg.nodes_per_core, cfg.n_pad
    BR = cfg.block_rows
    TPB = cfg.tiles_per_block
    cbg = cfg.cbg
    row1, row1w, row2 = cfg.row1, cfg.row1_w, cfg.row2
    F = cfg.f_in
    groups8 = [list(range(cfg.n_cores))]
    ccols_max = int((cfg.grp_icols + 16 * cfg.grp_chunks).max())

    # ---- inputs ----
    xT = nc.dram_tensor("xT", [F, npad], BF16, kind="ExternalInput")
    W1ext = nc.dram_tensor("W1ext", [F, D1 + H], BF16, kind="ExternalInput")
    ck = min(128, D1)
    nk = D1 // ck
    W2ext = nc.dram_tensor("W2ext", [ck, nk * (C + 2)], BF16, kind="ExternalInput")
    Wout = nc.dram_tensor("Wout", [C, cfg.classes], BF16, kind="ExternalInput")
    b1 = nc.dram_tensor("b1", [1, D1], F32, kind="ExternalInput")
    b2 = nc.dram_tensor("b2", [1, C], F32, kind="ExternalInput")
    bout = nc.dram_tensor("bout", [1, cfg.classes], F32, kind="ExternalInput")
    iota_rep = nc.dram_tensor("iota_rep", [1, 128 * cbg], BF16, kind="ExternalInput")
    cstream = nc.dram_tensor("cstream", [128, cfg.total_ccols], I16, kind="ExternalInput")
    ad1s = nc.dram_tensor("ad1s", [128, cfg.total_chunks * H], BF16,
                          kind="ExternalInput")
    ncnt = nc.dram_tensor("ncnt", [1, NG * G * B + NG * 8], mybir.dt.int32,
                          kind="ExternalInput")
    dstloc_p = nc.dram_tensor("dstloc_p", [128, cfg.total_chunks], BF16, kind="ExternalInput")

    # ---- internal / output ----
    tab1 = [nc.dram_tensor(f"tab1_{b}", [BR, row1], BF16) for b in range(B)]
    tab2_sh = nc.dram_tensor("tab2_sh", [npc, row2], BF16)
    tab2 = nc.dram_tensor("tab2", [npad, row2], BF16, addr_space="Shared")
    out = nc.dram_tensor("out", [npc, cfg.classes], F32, kind="ExternalOutput")

    with tile.TileContext(nc, num_cores=cfg.n_cores) as tc, ExitStack() as ctx:
        consts = ctx.enter_context(tc.tile_pool(name="consts", bufs=1))

        # resident constants
        W1e_sb = consts.tile([F, D1 + H], BF16)
        nc.sync.dma_start(out=W1e_sb, in_=W1ext[:])
        W2e_sb = consts.tile([ck, nk, C + 2], BF16)
        nc.sync.dma_start(out=W2e_sb, in_=W2ext[:].rearrange("p (a c) -> p a c", a=nk))
        Wout_sb = consts.tile([C, cfg.classes], BF16)
        nc.sync.dma_start(out=Wout_sb, in_=Wout[:])
        b1_sb = consts.tile([128, D1], F32)
        nc.sync.dma_start(out=b1_sb, in_=bass.AP(
            tensor=b1.ap().tensor, offset=0, ap=[[0, 128], [1, D1]]))
        b2_sb = consts.tile([128, C], F32)
        nc.sync.dma_start(out=b2_sb, in_=bass.AP(
            tensor=b2.ap().tensor, offset=0, ap=[[0, 128], [1, C]]))
        bout_sb = consts.tile([128, cfg.classes], F32)
        nc.sync.dma_start(out=bout_sb, in_=bass.AP(
            tensor=bout.ap().tensor, offset=0, ap=[[0, 128], [1, cfg.classes]]))
        iota_sb = consts.tile([128, 128 * cbg], BF16)
        nc.sync.dma_start(out=iota_sb, in_=bass.AP(
            tensor=iota_rep.ap().tensor, offset=0, ap=[[0, 128], [1, 128 * cbg]]))
        ident_sb = consts.tile([128, 128], BF16)
        make_identity(nc, ident_sb)
        zero_b = consts.tile([128, 1], F32)
        nc.vector.memset(zero_b, 0.0)
        mneg1_b = consts.tile([128, 1], F32)
        nc.vector.memset(mneg1_b, -cfg.m1)
        mneg2_b = consts.tile([128, 1], F32)
        nc.vector.memset(mneg2_b, -cfg.m2)
        ncnt_sb = consts.tile([1, NG * G * B + NG * 8], mybir.dt.int32)
        nc.sync.dma_start(out=ncnt_sb, in_=ncnt[:])
        gcnt_regs = [nc.gpsimd.alloc_register(f"gcnt{i}")
                     for i in range(2 * G * B)]
        adcnt_regs = [nc.gpsimd.alloc_register(f"adcnt{i}") for i in range(16)]
        dlp_sb = consts.tile([128, cfg.total_chunks], BF16)
        nc.sync.dma_start(out=dlp_sb, in_=dstloc_p[:])
        zs_buf = consts.tile([128, T, cfg.classes], BF16)
        ss_buf = consts.tile([128, T], F32)
        nc.vector.memset(zs_buf, 0.0)
        nc.vector.memset(ss_buf, 1.0)

        for _rep in range(cfg.repeat):
            # ------- phase A: replicated dense; full table1, per block -------
            # rows are batched 8 tiles per DMA (one strided write) to keep
            # the HWDGE/SP queue off the critical path.
            RB = 8
            with tc.tile_pool(name="phA", bufs=2) as pa, \
                 tc.tile_pool(name="phA_x", bufs=2) as pax, \
                 tc.tile_pool(name="phA_ps", bufs=4, space="PSUM") as pap:
                for b in range(B):
                    xT_sb = pax.tile([F, BR], BF16, tag="xTc")
                    nc.sync.dma_start(out=xT_sb, in_=xT[:, b * BR:(b + 1) * BR])
                    for t0 in range(0, TPB, RB):
                        rows = pa.tile([128, RB, row1w], BF16, tag="rowA")
                        for j in range(RB):
                            t = t0 + j
                            ps = pap.tile([128, D1 + H], F32, tag="psA")
                            nc.tensor.matmul(out=ps,
                                             lhsT=xT_sb[:, t * 128:(t + 1) * 128],
                                             rhs=W1e_sb[:], start=True, stop=True)
                            if j % 2 == 0:
                                nc.scalar.copy(rows[:, j, :], ps[:])
                            else:
                                nc.vector.tensor_copy(rows[:, j, :], ps[:])
                        nc.sync.dma_start(
                            out=bass.AP(tensor=tab1[b].ap().tensor,
                                        offset=t0 * 128 * row1,
                                        ap=[[row1, 128], [128 * row1, RB],
                                            [1, row1w]]),
                            in_=rows[:])

            # ---------------- shared edge pipeline ----------------
            def edge_phase(block_ap, ad_src, rowN, DN, HN, adoff, mneg_bias,
                           post_tile):
                """block_ap(b) -> gather source AP for block b.
                ad_src: None (layer 1: a_d from cstream) or (tensor, row cols)
                for a local dst-indexed gather."""
                with tc.tile_pool(name="phB_g", bufs=2) as pgath, \
                     tc.tile_pool(name="phB_ga", bufs=2) as pgad, \
                     tc.tile_pool(name="phB_ix", bufs=2) as pidx, \
                     tc.tile_pool(name="phB_S", bufs=2) as pS, \
                     tc.tile_pool(name="phB_m", bufs=2) as pmsg, \
                     tc.tile_pool(name="phB_ps", bufs=2, space="PSUM") as pbp, \
                     tc.tile_pool(name="phB_ep", bufs=2) as pep, \
                     tc.tile_pool(name="phB_ps2", bufs=2, space="PSUM") as pbp2:
                    for gi in range(NG):
                        cbgi = int(cfg.grp_chunks[gi])
                        choff = int(cfg.grp_choff[gi])
                        gioff = int(cfg.grp_ioff[gi])
                        gicols = int(cfg.grp_icols[gi])
                        gccols = gicols + 8 * cbgi
                        cs = pidx.tile([128, ccols_max], I16, tag="cs")
                        nc.sync.dma_start(out=cs[:, 0:gccols],
                                          in_=cstream[:, gioff:gioff + gccols])
                        if ad_src is None:
                            a1t = pidx.tile([128, cbg * H], BF16, tag="a1t")
                            nc.sync.dma_start(
                                out=a1t[:, 0:cbgi * H],
                                in_=ad1s[:, choff * H:(choff + cbgi) * H])
                        g = pgath.tile([128, cbg, rowN], BF16, tag="gath")
                        if gi < 2:
                            nc.vector.memset(g, 0.0)
                        if ad_src is not None:
                            adtab, adrow = ad_src
                            gad = pgad.tile([128, cbg, adrow], BF16, tag="gad")
                            for j, p0 in enumerate(range(0, cbgi, 8)):
                                pc = min(8, cbgi - p0)
                                ci = NG * G * B + gi * 8 + j
                                areg = adcnt_regs[(gi % 2) * 8 + j]
                                nc.gpsimd.reg_load(areg, ncnt_sb[0:1, ci:ci + 1])
                                nc.gpsimd.dma_gather(
                                    gad[:, p0:p0 + pc, :], adtab[:],
                                    cs[:, gicols + p0 * 8: gicols + (p0 + pc) * 8],
                                    pc * 128, areg, adrow, elem_step=adrow,
                                    queue_num=j % nc.num_swdge_queues)
                            ad_ap = gad[:, 0:cbgi, adoff:adoff + HN]
                        else:
                            ad_ap = a1t[:, 0:cbgi * H].rearrange(
                                "p (c h) -> p c h", h=H)
                        for b in range(B):
                            coff = int(cfg.call_coff[gi, b])
                            for ti in range(G):
                                t = gi * G + ti
                                cap = int(cfg.c_tb[t, b]) * 128
                                i0 = int(cfg.call_ioff[gi, ti, b]) - gioff
                                ci = (gi * G + ti) * B + b
                                reg = gcnt_regs[(gi % 2) * G * B + ti * B + b]
                                nc.gpsimd.reg_load(reg, ncnt_sb[0:1, ci:ci + 1])
                                nc.gpsimd.dma_gather(
                                    g[:, coff:coff + cap // 128, :],
                                    block_ap(b),
                                    cs[:, i0:i0 + cap // 16],
                                    cap, reg, rowN, elem_step=rowN,
                                    queue_num=b % nc.num_swdge_queues)
                                coff += cap // 128
                        # one-hot S[e, d, c] (chunk innermost; 2x mode)
                        S = pS.tile([128, 128, cbg], BF16, tag="S")
                        nc.vector.tensor_tensor(
                            out=S[:, :, :cbgi],
                            in0=iota_sb[:].rearrange("p (q c) -> p q c", q=128)[:, :, :cbgi],
                            in1=dlp_sb[:, choff:choff + cbgi].unsqueeze(1).broadcast_to(
                                [128, 128, cbgi]),
                            op=mybir.AluOpType.is_equal)
                        # es = a_s[src] + a_d[dst] (+ leaky)
                        es = pep.tile([128, cbg, HN], F32, tag="es")
                        nc.vector.tensor_tensor(out=es[:, :cbgi, :],
                                                in0=ad_ap,
                                                in1=g[:, :cbgi, DN:DN + HN],
                                                op=mybir.AluOpType.add)
                        nc.vector.scalar_tensor_tensor(
                            out=es[:, :cbgi, :], in0=es[:, :cbgi, :],
                            scalar=cfg.neg_slope, in1=es[:, :cbgi, :],
                            op0=mybir.AluOpType.mult, op1=mybir.AluOpType.max)
                        # msgee = [g * ee (head-minor broadcast) | ee]
                        ee = pep.tile([128, cbg, HN], BF16, tag="ee")
                        nc.scalar.activation(ee[:, :cbgi, :], es[:, :cbgi, :],
                                             mybir.ActivationFunctionType.Exp,
                                             bias=mneg_bias[:], scale=1.0)
                        msgee = pmsg.tile([128, cbg, DN + HN], BF16, tag="msgee")
                        nc.vector.tensor_copy(msgee[:, :cbgi, DN:DN + HN],
                                              ee[:, :cbgi, :])
                        if HN > 1:
                            nc.vector.tensor_tensor(
                                out=msgee[:, :cbgi, 0:DN].rearrange(
                                    "p c (w h) -> p c w h", h=HN),
                                in0=g[:, :cbgi, 0:DN].rearrange(
                                    "p c (w h) -> p c w h", h=HN),
                                in1=ee[:, :cbgi, :].unsqueeze(2).broadcast_to(
                                    [128, cbgi, DN // HN, HN]),
                                op=mybir.AluOpType.mult)
                        else:
                            nc.vector.tensor_tensor(
                                out=msgee[:, :cbgi, 0:DN],
                                in0=g[:, :cbgi, 0:DN],
                                in1=ee[:, :cbgi, :].broadcast_to(
                                    [128, cbgi, DN]),
                                op=mybir.AluOpType.mult)
                        # aggregate per tile over its chunk runs
                        for ti in range(G):
                            t = gi * G + ti
                            runs = cfg.tile_runs[t]
                            chunks = [p for (pos, cnt) in runs
                                      for p in range(pos, pos + cnt)]
                            psG = pbp.tile([128, DN + HN], F32, tag="psG")
                            for jj, j in enumerate(chunks):
                                nc.tensor.matmul(out=psG, lhsT=S[:, :, j],
                                                 rhs=msgee[:, j, :],
                                                 start=(jj == 0),
                                                 stop=(jj == len(chunks) - 1))
                            post_tile(t, psG, pep, pbp2)

            # ---- layer-1 post-tile: divide, +b1, elu, h2 fold ----
            def post1(t, psG, pep, pps):
                den = pep.tile([128, H], F32, tag="den")
                nc.vector.tensor_scalar_add(den, psG[:, D1:D1 + H], 1e-30)
                rden = pep.tile([128, H], F32, tag="rden")
                nc.vector.reciprocal(rden, den[:])
                z = pep.tile([128, D1], F32, tag="z")
                nc.vector.tensor_tensor(
                    out=z[:].rearrange("p (w h) -> p w h", h=H),
                    in0=psG[:, 0:D1].rearrange("p (w h) -> p w h", h=H),
                    in1=rden[:].unsqueeze(1).broadcast_to([128, C, H]),
                    op=mybir.AluOpType.mult)
                nc.vector.tensor_tensor(out=z, in0=z[:], in1=b1_sb[:],
                                        op=mybir.AluOpType.add)
                zp = pep.tile([128, D1], F32, tag="zp")
                nc.vector.tensor_scalar_max(zp, z[:], 0.0)
                nc.vector.tensor_scalar_min(z, z[:], 0.0)  # in place
                em = pep.tile([128, D1], F32, tag="em")
                nc.scalar.activation(em, z[:], mybir.ActivationFunctionType.Exp,
                                     bias=zero_b[:])
                elu_bf = pep.tile([128, D1], BF16, tag="elubf")
                nc.vector.scalar_tensor_tensor(
                    out=elu_bf, in0=em[:], scalar=-1.0, in1=zp[:],
                    op0=mybir.AluOpType.add, op1=mybir.AluOpType.add)
                # h2 = eluT @ W2ext
                eluT = pep.tile([ck, nk, 128], BF16, tag="eluT")
                for kk in range(nk):
                    psT = pps.tile([ck, 128], BF16, tag="psmisc")
                    nc.tensor.transpose(psT, elu_bf[:, kk * ck:(kk + 1) * ck], ident_sb[:])
                    nc.scalar.copy(eluT[:, kk, :], psT[:])
                psH2 = pps.tile([128, C + 2], F32, tag="psmisc")
                for kk in range(nk):
                    nc.tensor.matmul(out=psH2, lhsT=eluT[:, kk, :], rhs=W2e_sb[:, kk, :],
                                     start=(kk == 0), stop=(kk == nk - 1))
                row = pep.tile([128, row2], BF16, tag="rowC")
                if row2 > C + 2:
                    nc.vector.memset(row[:, C + 2:], 0.0)
                nc.scalar.copy(row[:, 0:C + 2], psH2[:])
                nc.sync.dma_start(out=tab2_sh[t * 128:(t + 1) * 128, :], in_=row[:])

            if cfg.ablate == 'phaseA':
                with tc.tile_pool(name="dummy", bufs=1) as pd:
                    zt = pd.tile([128, T, cfg.classes], F32)
                    nc.vector.memset(zt, 0.0)
                    nc.sync.dma_start(
                        out=bass.AP(tensor=out.ap().tensor, offset=0,
                                    ap=[[cfg.classes, 128],
                                        [128 * cfg.classes, T],
                                        [1, cfg.classes]]),
                        in_=zt[:])
            if cfg.ablate != 'phaseA':
                edge_phase(lambda b: tab1[b][:], None, row1, D1, H, 0,
                           mneg1_b, post1)

            if cfg.ablate in ('phaseA', 'e1'):
                pass
            elif cfg.ablate == "nocoll":
                nc.sync.dma_start(out=tab2[0:npc, :], in_=tab2_sh[:])
            else:
                nc.gpsimd.collective_compute(
                    "AllGather", mybir.AluOpType.bypass, groups8,
                    ins=[tab2_sh[:]], outs=[tab2[:]])

            # ---- layer-2 post-tile: divide, +b2, elu, head, partial smax ----
            def post2(t, psG, pep, pps):
                den = pep.tile([128, 1], F32, tag="den2")
                nc.vector.tensor_scalar_add(den, psG[:, C:C + 1], 1e-30)
                rden = pep.tile([128, 1], F32, tag="rden2")
                nc.vector.reciprocal(rden, den[:])
                z = pep.tile([128, C], F32, tag="z2")
                nc.vector.tensor_scalar(out=z, in0=psG[:, 0:C], scalar1=rden[:],
                                        scalar2=None, op0=mybir.AluOpType.mult)
                nc.vector.tensor_tensor(out=z, in0=z[:], in1=b2_sb[:],
                                        op=mybir.AluOpType.add)
                zp = pep.tile([128, C], F32, tag="zp2")
                nc.vector.tensor_scalar_max(zp, z[:], 0.0)
                nc.vector.tensor_scalar_min(z, z[:], 0.0)  # in place
                em = pep.tile([128, C], F32, tag="em2")
                nc.scalar.activation(em, z[:], mybir.ActivationFunctionType.Exp,
                                     bias=zero_b[:])
                h3 = pep.tile([128, C], BF16, tag="h3")
                nc.vector.scalar_tensor_tensor(
                    out=h3, in0=em[:], scalar=-1.0, in1=zp[:],
                    op0=mybir.AluOpType.add, op1=mybir.AluOpType.add)
                psT = pps.tile([C, 128], BF16, tag="psmisc2")
                nc.tensor.transpose(psT, h3[:], ident_sb[:])
                h3T = pep.tile([C, 128], BF16, tag="h3T")
                nc.scalar.copy(h3T, psT[:])
                psL = pps.tile([128, cfg.classes], F32, tag="psmisc2")
                nc.tensor.matmul(out=psL, lhsT=h3T[:], rhs=Wout_sb[:], start=True, stop=True)
                z3 = pep.tile([128, cfg.classes], F32, tag="z3")
                nc.vector.tensor_tensor(out=z3, in0=psL[:], in1=bout_sb[:],
                                        op=mybir.AluOpType.add)
                mx = pep.tile([128, 1], F32, tag="mx")
                nc.vector.tensor_reduce(mx, z3[:], axis=mybir.AxisListType.X,
                                        op=mybir.AluOpType.max)
                nc.vector.tensor_scalar(out=zs_buf[:, t, :], in0=z3[:], scalar1=mx[:],
                                        scalar2=None, op0=mybir.AluOpType.subtract)
                es3 = pep.tile([128, cfg.classes], F32, tag="es3")
                nc.scalar.activation(es3, zs_buf[:, t, :],
                                     mybir.ActivationFunctionType.Exp,
                                     bias=zero_b[:], accum_out=ss_buf[:, t:t + 1])

            if cfg.ablate not in ('phaseA', 'e1'):
                edge_phase(lambda b: tab2[b * BR:(b + 1) * BR, :], (tab2_sh, row2),
                           row2, C, 1, C + 1, mneg2_b, post2)

            # ---- batched log-softmax tail: one Ln + one subtract + one DMA ----
            with tc.tile_pool(name="tail", bufs=1) as pt:
                if cfg.ablate == 'phaseA':
                    pt = pt  # pools still open/close
                
                lg = pt.tile([128, T], F32)
                nc.scalar.activation(lg, ss_buf[:],
                                     mybir.ActivationFunctionType.Ln,
                                     bias=zero_b[:])
                oo = pt.tile([128, T, cfg.classes], F32)
                nc.vector.tensor_tensor(
                    out=oo, in0=zs_buf[:],
                    in1=lg[:].unsqueeze(2).broadcast_to([128, T, cfg.classes]),
                    op=mybir.AluOpType.subtract)
                nc.sync.dma_start(
                    out=bass.AP(tensor=out.ap().tensor, offset=0,
                                ap=[[cfg.classes, 128],
                                    [128 * cfg.classes, T],
                                    [1, cfg.classes]]),
                    in_=oo[:])

    nc.compile()
    return nc


# ============================ public entry point ============================

_CACHE = {}


def kernel(**inputs):
    import numpy as np
    cfg = Cfg()
    hd = prep_host_data(cfg, inputs)
    key = ("gat_v4", cfg.n_pad, cfg.total_chunks, cfg.total_ccols,
           tuple(cfg.c_tb.reshape(-1).tolist()))
    nc = _CACHE.get(key)
    if nc is None:
        nc = build_program(cfg)
        _CACHE.clear()
        _CACHE[key] = nc
    from concourse.bass_utils import run_bass_kernel_spmd
    res = None
    last_err = None
    for _attempt in range(3):
        try:
            res = run_bass_kernel_spmd(nc, hd.per_core, list(range(cfg.n_cores)))
            break
        except Exception as e:  # device may need one reconnect after a crash
            last_err = e
    if res is None:
        raise last_err
    full = np.concatenate([res.results[i]["out"] for i in range(cfg.n_cores)],
                          axis=0)
    out = full[hd.perm[:cfg.n_nodes]]
    return np.ascontiguousarray(out.astype(np.float32))
